# revision 30
# baseline (speedup 1.0000x reference)
"""TRN2 Bass kernel for nn_DecoderLayer_47175920779446.

Full decoder layer: qkv (mul-bias) -> 16-head attention -> +res -> LN ->
FFN(relu, mul-bias) -> +res -> LN, on x[2, 2048, 1024] fp32.

Sharding (8 cores): attention is sharded by (batch, 4 heads): core c handles
batch c//4, heads 4*(c%4)..4*(c%4)+3 over all 2048 tokens of its batch.
FFN/LN are sharded by strided 64-token blocks: core c owns blocks
{t//64 == 8*m + c} of BOTH batches (256+256 tokens). Attention runs
query-group-outer; after each group g finishes (4 heads), a small fp16
AllToAll chunk (8 x [8, 64, 256]) reshards that group's attention output,
and the full FFN chain for that 128-token tile (LN1 -> FFN1 -> FFN2 -> LN2)
runs interleaved under the next attention group's matmuls.

Precision: scores run in fp16 (11-bit mantissa, matching the fp16 projection
noise floor ~2e-4*sigma): S = q16*k16 + m_hat (K=65 with a fused bias row).
V/P/FFN run fp16; LN stats and residual sums run fp32 where it matters.
"""
import contextlib
import numpy as np

import concourse.bass as bass
import concourse.tile as tile
from concourse import bacc, mybir
from concourse.bass_utils import run_bass_kernel_spmd
from concourse.bass_interp import get_hw_module
from concourse.masks import make_identity

H, NH, HD, FF = 1024, 16, 64, 4096
B, T = 2, 2048
EPS = 1e-6
NCORES = 8
HPC = NH // 4          # 4 heads per core
TOK = (B * T) // NCORES  # 512 tokens per core through the FFN
NKC = T // 128         # 16 key chunks
NG = T // 512          # 4 query groups
KCH = H // 128         # 8 contraction chunks for qkv
f32, f32r, bf16 = mybir.dt.float32, mybir.dt.float32r, mybir.dt.bfloat16
f16 = mybir.dt.float16
AF = mybir.ActivationFunctionType
ALU = mybir.AluOpType


def _build_program(sim_single=False):
    nc = bacc.Bacc("TRN2", target_bir_lowering=False, debug=False,
                   num_devices=1 if sim_single else NCORES)
    ap = {}
    ap["xT"] = nc.dram_tensor("xT", [H, T], f16, kind="ExternalInput").ap()
    ap["xres"] = nc.dram_tensor("xres", [TOK, H], f16, kind="ExternalInput").ap()
    for w in ("wq", "wk", "wv"):
        ap[w] = nc.dram_tensor(w, [H, 4 * HD], f16, kind="ExternalInput").ap()
    ap["w1"] = nc.dram_tensor("w1", [H, FF], f16, kind="ExternalInput").ap()
    ap["w2"] = nc.dram_tensor("w2", [FF, H], f16, kind="ExternalInput").ap()
    ap["lnw"] = nc.dram_tensor("lnw", [4, H], f16, kind="ExternalInput").ap()
    out_ap = nc.dram_tensor("out", [TOK, H], f16, kind="ExternalOutput").ap()

    with tile.TileContext(nc) as tc:
        ctx = contextlib.ExitStack()
        with ctx:
            const = ctx.enter_context(tc.tile_pool(name="const", bufs=1))
            dram = ctx.enter_context(tc.tile_pool(name="dram", bufs=1, space="DRAM"))

            identh = const.tile([128, 128], f16)
            make_identity(nc, identh[:])

            # per-group a2a chunks: slot r holds 64-token block (t//64 == 8g+r)
            # of this core's batch, its 4 heads (256 dims), fp16.
            a2a_in = [dram.tile([NCORES, 64, 4 * HD], f16, name=f"a2a_in{g}")
                      for g in range(NG)]
            a2a_out = [dram.tile([NCORES, 64, 4 * HD], f16, name=f"a2a_out{g}")
                       for g in range(NG)]

            # score/softmax operand pools (live through all attention units)
            qk = ctx.enter_context(tc.tile_pool(name="qk", bufs=1))
            sb = ctx.enter_context(tc.tile_pool(name="sb", bufs=3))
            small = ctx.enter_context(tc.tile_pool(name="small", bufs=4))
            psn = ctx.enter_context(tc.tile_pool(name="psn", bufs=2, space="PSUM"))
            pss = ctx.enter_context(tc.tile_pool(name="pss", bufs=2, space="PSUM"))
            pso = ctx.enter_context(tc.tile_pool(name="pso", bufs=1, space="PSUM"))
            psm = ctx.enter_context(tc.tile_pool(name="psm", bufs=2, space="PSUM"))

            til_q, til_k = {}, {}
            for h in range(HPC):
                til_q[h] = qk.tile([65, T], f16, name=f"til_q{h}", tag="tq", bufs=HPC)
                til_k[h] = qk.tile([65, T], f16, name=f"til_k{h}", tag="tk", bufs=HPC)
                nc.gpsimd.memset(til_k[h][64:65, :], 1.0)
            vn = []
            for kc in range(NKC):
                v = qk.tile([128, HPC, 65], f16, name=f"vn{kc}", tag="vn", bufs=NKC)
                nc.gpsimd.memset(v[:, :, 64:65], 1.0)
                vn.append(v)

            # ---------------- QKV projections (own scope: frees w/x tiles) --
            qctx = contextlib.ExitStack()
            with qctx:
                wpool = qctx.enter_context(tc.tile_pool(name="wpool", bufs=1))
                xgp = qctx.enter_context(tc.tile_pool(name="xgp", bufs=4))

                w_sb = {}
                for w in ("wk", "wq", "wv"):
                    w_sb[w] = wpool.tile([128, KCH, 4 * HD], f16, name=f"sb_{w}")
                    nc.sync.dma_start(
                        w_sb[w][:], ap[w].rearrange("(a p) c -> p a c", p=128))
                xgs = []
                for g in range(NG):
                    gsl = slice(512 * g, 512 * (g + 1))
                    xg = xgp.tile([128, KCH, 512], f16, name=f"xg{g}", tag="xg", bufs=4)
                    nc.sync.dma_start(
                        xg[:], ap["xT"].rearrange("(a p) t -> p a t", p=128)[:, :, gsl])
                    xgs.append(xg)

                def proj_pass(name, til, g):
                    gsl = slice(512 * g, 512 * (g + 1))
                    for hp in range(2):  # head pairs
                        p = pss.tile([128, 512], f32, tag="st", name="pqk")
                        for a in range(KCH):
                            nc.tensor.matmul(
                                p[:], w_sb[name][:, a, 128 * hp:128 * (hp + 1)],
                                xgs[g][:, a, :], start=(a == 0), stop=(a == KCH - 1))
                        for hl in range(2):
                            h = 2 * hp + hl
                            rows = slice(64 * hl, 64 * (hl + 1))
                            nc.gpsimd.tensor_copy(til[h][0:64, gsl], p[rows, :])

                for g in range(NG):
                    proj_pass("wk", til_k, g)
                for g in range(NG):
                    proj_pass("wq", til_q, g)
                    for tt in range(4):  # V natural per token tile
                        kc = 4 * g + tt
                        p = pss.tile([128, 4 * HD], f32, tag="st", name="pv")
                        for a in range(KCH):
                            nc.tensor.matmul(
                                p[:], xgs[g][:, a, 128 * tt:128 * (tt + 1)],
                                w_sb["wv"][:, a, :], start=(a == 0), stop=(a == KCH - 1))
                        nc.gpsimd.tensor_copy(
                            vn[kc][:, :, 0:64],
                            p[:].rearrange("p (h d) -> p h d", h=HPC))

            # ---------------- FFN pools (reuse qkv space; weights stream
            # during attention) ----------------
            w1p = ctx.enter_context(tc.tile_pool(name="w1p", bufs=1))
            w2p = ctx.enter_context(tc.tile_pool(name="w2p", bufs=1))
            fsb = ctx.enter_context(tc.tile_pool(name="fsb", bufs=2))
            o1p = ctx.enter_context(tc.tile_pool(name="o1p", bufs=1))
            fsm = ctx.enter_context(tc.tile_pool(name="fsm", bufs=4))
            psf = ctx.enter_context(tc.tile_pool(name="psf", bufs=1, space="PSUM"))

            w1t = w1p.tile([128, KCH, FF], f16, name="w1t")
            nc.sync.dma_start(
                w1t[:], ap["w1"].rearrange("(a p) f -> p a f", p=128))
            w2t = w2p.tile([128, FF // 128, H], f16, name="w2t")
            nc.sync.dma_start(
                w2t[:], ap["w2"].rearrange("(a p) o -> p a o", p=128))

            lnbc = {}
            for i, nm in enumerate(("g1", "b1", "g2", "b2")):
                lnbc[nm] = o1p.tile([128, H], f16, name=f"ln_{nm}", tag="lnbc", bufs=4)
                nc.sync.dma_start(
                    lnbc[nm][:], ap["lnw"][i, :].partition_broadcast(128))


            # ---------------- attention stages ----------------
            def stage_a1(h, g):
                # natural-S matmuls + DVE max reduces
                mstage = small.tile([128, 4], f16, tag="mstage", name="mstage", bufs=2)
                for qt in range(4):
                    qsl = slice(512 * g + 128 * qt, 512 * g + 128 * (qt + 1))
                    negmax = []
                    for half in range(4):
                        sn = psn.tile([128, 512], f32, name="sn")
                        ks = slice(512 * half, 512 * (half + 1))
                        nc.tensor.matmul(
                            sn[:], til_q[h][0:64, qsl], til_k[h][0:64, ks],
                            start=True, stop=True)
                        nm = small.tile([128, 1], f32, tag="nm", name="nm", bufs=8)
                        nc.vector.tensor_reduce(
                            nm[:], sn[:], axis=mybir.AxisListType.X,
                            op=ALU.max, negate=True)
                        negmax.append(nm)
                    nc.vector.tensor_tensor(
                        negmax[0][:], negmax[0][:], negmax[1][:], ALU.min)
                    nc.vector.tensor_tensor(
                        negmax[2][:], negmax[2][:], negmax[3][:], ALU.min)
                    nc.vector.tensor_tensor(
                        mstage[:, qt:qt + 1], negmax[0][:], negmax[2][:], ALU.min)
                return mstage

            def stage_a2(h, g, mstage):
                # emitted a period later so the PE transpose never waits on DVE
                for qt in range(4):
                    qsl = slice(512 * g + 128 * qt, 512 * g + 128 * (qt + 1))
                    mt = psm.tile([1, 128], f16, tag="mt", name="mt")
                    nc.tensor.transpose(mt[:], mstage[:, qt:qt + 1], identh[:])
                    nc.gpsimd.tensor_copy(til_q[h][64:65, qsl], mt[:])

            def stage_b(h, g):
                gsl = slice(512 * g, 512 * (g + 1))
                o_acc = pso.tile([65, 512], f32, name="o_acc")
                pts = {}
                PVLAG = 2

                def pv(kc):
                    nc.tensor.matmul(o_acc[:], vn[kc][:, h, :], pts.pop(kc)[:],
                                     start=(kc == 0), stop=(kc == NKC - 1))

                for kc in range(NKC):
                    ksl = slice(128 * kc, 128 * (kc + 1))
                    st = pss.tile([128, 512], f32, tag="st", name="st")
                    nc.tensor.matmul(st[:], til_k[h][0:65, ksl],
                                     til_q[h][0:65, gsl], start=True, stop=True)
                    pt = sb.tile([128, 512], f16, tag="pt", name="pt", bufs=3)
                    nc.scalar.activation(pt[:], st[:], AF.Exp)
                    pts[kc] = pt
                    if kc >= PVLAG:
                        pv(kc - PVLAG)
                for kc in range(NKC - PVLAG, NKC):
                    pv(kc)
                ot = sb.tile([65, 512], f16, tag="ot", name="ot", bufs=2)
                nc.gpsimd.tensor_copy(ot[:], o_acc[:])
                # transpose to natural, scale by 1/denom, ship to a2a chunk g:
                # token rows 0:64 -> slot 2*tt, rows 64:128 -> slot 2*tt+1
                for tt in range(4):
                    op_ = psm.tile([128, 65], f16, tag="mt", name="opt")
                    nc.tensor.transpose(
                        op_[:], ot[0:65, 128 * tt:128 * (tt + 1)],
                        identh[0:65, 0:65])
                    rc = small.tile([128, 1], f32, tag="rc", name="rc")
                    nc.vector.reciprocal(rc[:], op_[:, 64:65])
                    ob = sb.tile([128, HD], f16, tag="ob", name="ob", bufs=2)
                    nc.gpsimd.tensor_scalar_mul(ob[:], op_[:, 0:64], rc[:])
                    nc.sync.dma_start(
                        a2a_in[g][2 * tt:2 * tt + 2, :, 64 * h:64 * (h + 1)],
                        ob[:].rearrange("(s p) d -> s p d", s=2))

            # ---------------- per-chunk FFN chain ----------------
            def layer_norm_to(dst, src, g_bc, b_bc, work):
                """dst = gamma*(src-mean)/(std_unbiased+EPS)+beta, [128,H]."""
                stats = fsm.tile([128, 2, 6], f32, tag="stats", name="stats")
                for hf in range(2):
                    nc.vector.bn_stats(stats[:, hf, :],
                                       src[:, 512 * hf:512 * (hf + 1)])
                mv = fsm.tile([128, 2], f32, tag="mv", name="mv")
                nc.vector.bn_aggr(mv[:], stats[:])
                sd = fsm.tile([128, 1], f32, tag="sd", name="sd")
                nc.scalar.activation(sd[:], mv[:, 1:2], AF.Sqrt,
                                     scale=float(H) / (H - 1))
                nc.vector.tensor_scalar_add(sd[:], sd[:], EPS)
                rs = fsm.tile([128, 1], f32, tag="rs", name="rs")
                nc.vector.reciprocal(rs[:], sd[:])
                nc.vector.tensor_scalar(out=work[:], in0=src[:],
                                        scalar1=mv[:, 0:1], scalar2=rs[:],
                                        op0=ALU.subtract, op1=ALU.mult)
                nc.vector.tensor_mul(work[:], work[:], g_bc[:])
                nc.vector.tensor_add(dst[:], work[:], b_bc[:])

            def ffn_chunk(g):
                if sim_single:
                    nc.sync.dma_start(a2a_out[g][:], a2a_in[g][:])
                else:
                    nc.gpsimd.collective_compute(
                        "AllToAll", ALU.bypass,
                        replica_groups=[list(range(NCORES))],
                        ins=[a2a_in[g].opt()], outs=[a2a_out[g].opt()])
                # assemble attention-out tile: rows 0:64 batch-0 block,
                # rows 64:128 batch-1 block; src r covers dims 256*(r%4)
                at = fsb.tile([128, H], f16, tag="ta", name="at", bufs=1)
                for r in range(NCORES):
                    rh, cs = (r // 4) * 64, 256 * (r % 4)
                    nc.sync.dma_start(at[rh:rh + 64, cs:cs + 256],
                                      a2a_out[g][r, :, :])
                xr = fsm.tile([128, H], f16, tag="xr", name="xr", bufs=1)
                nc.sync.dma_start(xr[:], ap["xres"][128 * g:128 * (g + 1), :])
                nc.vector.tensor_add(at[:], at[:], xr[:])
                out1 = fsb.tile([128, H], f16, tag="o1", name="out1")
                wk_ = fsb.tile([128, H], f16, tag="td", name="wk_", bufs=1)
                layer_norm_to(out1, at, lnbc["g1"], lnbc["b1"], wk_)
                # transpose LN1 out for the H-contraction of FFN1
                out1T = fsb.tile([128, KCH, 128], f16, tag="o1T", name="out1T",
                                 bufs=1)
                for a in range(KCH):
                    tp = psm.tile([128, 128], f16, tag="mt", name="tp")
                    nc.tensor.transpose(
                        tp[:], out1[:, 128 * a:128 * (a + 1)], identh[:])
                    nc.gpsimd.tensor_copy(out1T[:, a, :], tp[:])
                # FFN1: ht[f, t] = relu(W1^T x out1T) for this 128-token tile
                ht = fsb.tile([128, FF // 128, 128], f16, tag="ht", name="ht",
                              bufs=1)
                for ft in range(FF // 128):
                    hp_ = psf.tile([128, 128], f32, tag="hp", name="hp", bufs=1)
                    for a in range(KCH):
                        nc.tensor.matmul(
                            hp_[:], w1t[:, a, 128 * ft:128 * (ft + 1)],
                            out1T[:, a, :], start=(a == 0), stop=(a == KCH - 1))
                    nc.gpsimd.tensor_scalar_max(ht[:, ft, :], hp_[:], 0.0)
                # FFN2 + fused LN2 tail
                h2 = fsb.tile([128, H], f16, tag="tc", name="h2", bufs=1)
                for oc in range(2):
                    acc = pss.tile([128, 512], f32, tag="st", name="o2acc")
                    for ft in range(FF // 128):
                        nc.tensor.matmul(
                            acc[:], ht[:, ft, :],
                            w2t[:, ft, 512 * oc:512 * (oc + 1)],
                            start=(ft == 0), stop=(ft == FF // 128 - 1))
                    nc.gpsimd.tensor_copy(h2[:, 512 * oc:512 * (oc + 1)], acc[:])
                nc.vector.tensor_add(h2[:], h2[:], out1[:])
                fin = fsb.tile([128, H], f16, tag="tb2", name="fin", bufs=1)
                wk2 = fsb.tile([128, H], f16, tag="td", name="wk2", bufs=1)
                layer_norm_to(fin, h2, lnbc["g2"], lnbc["b2"], wk2)
                nc.sync.dma_start(out_ap[128 * g:128 * (g + 1), :], fin[:])

            # ---------------- main loop: g-outer units, FFN chunk g emitted
            # two units after group g's last head ----------------
            units = [(h, g) for g in range(NG) for h in range(HPC)]
            FFN_LAG = 2  # units of cover between a2a chunk and its FFN chain
            LOOKAHEAD = 3
            mstages, done_a2 = {}, set()
            for k in range(min(LOOKAHEAD, len(units))):
                mstages[k] = stage_a1(*units[k])
                stage_a2(*units[k], mstages.pop(k))
                done_a2.add(k)
            for i, (h, g) in enumerate(units):
                j = i + LOOKAHEAD
                if j < len(units):
                    mstages[j] = stage_a1(*units[j])
                j2 = i + LOOKAHEAD - 1
                if j2 < len(units) and j2 not in done_a2:
                    stage_a2(*units[j2], mstages.pop(j2))
                    done_a2.add(j2)
                stage_b(h, g)
                gdone = i // HPC - 1 if (i % HPC) == FFN_LAG - 1 else None
                if gdone is not None and gdone >= 0:
                    ffn_chunk(gdone)
            for g in (NG - 2, NG - 1):
                ffn_chunk(g)

    nc.compile()
    if not sim_single:
        nc.m = get_hw_module(nc.m)
    return nc


_NC_CACHE = {}


def _get_program():
    if "nc" not in _NC_CACHE:
        _NC_CACHE["nc"] = _build_program()
    return _NC_CACHE["nc"]


def _prep_inputs(x, Wqkv, bqkv, W1, b1, W2, b2, gamma1, beta1, gamma2, beta2):
    """Host-side slicing/folding into per-core in_maps."""
    x = np.asarray(x, np.float32)
    Wqkv = np.asarray(Wqkv, np.float32)
    bqkv = np.asarray(bqkv, np.float32)
    d = np.arange(HD)
    hh = np.arange(NH)
    # qkv reshape in reference: [B,T,HD,3,NH] -> col = d*48 + k*16 + h
    cols = d[:, None, None] * (3 * NH) + np.arange(3)[None, :, None] * NH \
        + hh[None, None, :]
    Wq = Wqkv[:, cols[:, 0, :]] * (bqkv[cols[:, 0, :]] / np.sqrt(H))[None]
    Wk = Wqkv[:, cols[:, 1, :]] * bqkv[cols[:, 1, :]][None]
    Wv = Wqkv[:, cols[:, 2, :]] * bqkv[cols[:, 2, :]][None]
    # -> [H, HD, NH]; per-core head-major layout [H, 4*HD] (head-local major)
    Wq = np.transpose(Wq, (0, 2, 1)).astype(np.float16)  # [H, NH, HD]
    Wk = np.transpose(Wk, (0, 2, 1)).astype(np.float16)
    Wv = np.transpose(Wv, (0, 2, 1)).astype(np.float16)
    W1e = (np.asarray(W1, np.float32) * np.asarray(b1, np.float32)[None]) \
        .astype(np.float16)
    W2e = (np.asarray(W2, np.float32) * np.asarray(b2, np.float32)[None]) \
        .astype(np.float16)
    lnw = np.stack([gamma1, beta1, gamma2, beta2]).astype(np.float16)
    xT = [np.ascontiguousarray(x[b].T).astype(np.float16) for b in range(B)]
    in_maps = []
    for c in range(NCORES):
        b, grp = c // 4, c % 4
        heads = slice(4 * grp, 4 * grp + 4)
        # FFN tokens of core c: per group g, batch b': x[b', 512g+64c : +64]
        xres = np.empty((TOK, H), np.float16)
        for g in range(NG):
            t0 = 512 * g + 64 * c
            xres[128 * g:128 * g + 64] = x[0, t0:t0 + 64]
            xres[128 * g + 64:128 * g + 128] = x[1, t0:t0 + 64]
        in_maps.append({
            "xT": xT[b],
            "xres": xres,
            "wq": np.ascontiguousarray(Wq[:, heads, :].reshape(H, 4 * HD)),
            "wk": np.ascontiguousarray(Wk[:, heads, :].reshape(H, 4 * HD)),
            "wv": np.ascontiguousarray(Wv[:, heads, :].reshape(H, 4 * HD)),
            "w1": W1e, "w2": W2e, "lnw": lnw,
        })
    return in_maps


def kernel(x, Wqkv, bqkv, W1, b1, W2, b2, gamma1, beta1, gamma2, beta2,
           _trace=False):
    nc = _get_program()
    in_maps = _prep_inputs(x, Wqkv, bqkv, W1, b1, W2, b2,
                           gamma1, beta1, gamma2, beta2)
    res = run_bass_kernel_spmd(nc, in_maps, core_ids=list(range(NCORES)),
                               trace=_trace)
    out = np.empty((B, T, H), np.float32)
    for c in range(NCORES):
        oc = res.results[c]["out"]  # [512, H]: (g, batch-half, 64)
        for g in range(NG):
            t0 = 512 * g + 64 * c
            out[0, t0:t0 + 64] = oc[128 * g:128 * g + 64]
            out[1, t0:t0 + 64] = oc[128 * g + 64:128 * g + 128]
    if _trace:
        kernel.last_results = res
    return out


# revision 33
# speedup vs baseline: 1.1064x; 1.1064x over previous
"""TRN2 Bass kernel for nn_DecoderLayer_47175920779446.

Full decoder layer: qkv (mul-bias) -> 16-head attention -> +res -> LN ->
FFN(relu, mul-bias) -> +res -> LN, on x[2, 2048, 1024] fp32.

Sharding (8 cores): attention is sharded by (batch, 4 heads): core c handles
batch c//4, heads 4*(c%4)..4*(c%4)+3 over all 2048 tokens of its batch.
FFN/LN are sharded by strided 64-token blocks: core c owns blocks
{t//64 == 8*m + c} of BOTH batches (256+256 tokens). Attention runs
query-group-outer; after each group g finishes (4 heads), a small fp16
AllToAll chunk (8 x [8, 64, 256]) reshards that group's attention output,
and the full FFN chain for that 128-token tile (LN1 -> FFN1 -> FFN2 -> LN2)
runs interleaved under the next attention group's matmuls.

Precision: scores run in fp16 (11-bit mantissa, matching the fp16 projection
noise floor ~2e-4*sigma): S = q16*k16 + m_hat (K=65 with a fused bias row).
V/P/FFN run fp16; LN stats and residual sums run fp32 where it matters.
"""
import contextlib
import numpy as np

import concourse.bass as bass
import concourse.tile as tile
from concourse import bacc, mybir
from concourse.bass_utils import run_bass_kernel_spmd
from concourse.bass_interp import get_hw_module
from concourse.masks import make_identity

H, NH, HD, FF = 1024, 16, 64, 4096
B, T = 2, 2048
EPS = 1e-6
NCORES = 8
HPC = NH // 4          # 4 heads per core
TOK = (B * T) // NCORES  # 512 tokens per core through the FFN
NKC = T // 128         # 16 key chunks
NG = T // 512          # 4 query groups
KCH = H // 128         # 8 contraction chunks for qkv
f32, f32r, bf16 = mybir.dt.float32, mybir.dt.float32r, mybir.dt.bfloat16
f16 = mybir.dt.float16
AF = mybir.ActivationFunctionType
ALU = mybir.AluOpType


def _build_program(sim_single=False):
    nc = bacc.Bacc("TRN2", target_bir_lowering=False, debug=False,
                   num_devices=1 if sim_single else NCORES)
    ap = {}
    ap["xT"] = nc.dram_tensor("xT", [H, T], f16, kind="ExternalInput").ap()
    ap["xres"] = nc.dram_tensor("xres", [TOK, H], f16, kind="ExternalInput").ap()
    for w in ("wq", "wk", "wv"):
        ap[w] = nc.dram_tensor(w, [H, 4 * HD], f16, kind="ExternalInput").ap()
    ap["w1"] = nc.dram_tensor("w1", [H, FF], f16, kind="ExternalInput").ap()
    ap["w2"] = nc.dram_tensor("w2", [FF, H], f16, kind="ExternalInput").ap()
    ap["lnw"] = nc.dram_tensor("lnw", [4, H], f16, kind="ExternalInput").ap()
    out_ap = nc.dram_tensor("out", [TOK, H], f16, kind="ExternalOutput").ap()

    with tile.TileContext(nc) as tc:
        ctx = contextlib.ExitStack()
        with ctx:
            const = ctx.enter_context(tc.tile_pool(name="const", bufs=1))
            dram = ctx.enter_context(tc.tile_pool(name="dram", bufs=1, space="DRAM"))

            identh = const.tile([128, 128], f16)
            make_identity(nc, identh[:])

            # per-group a2a chunks: slot r holds 64-token block (t//64 == 8g+r)
            # of this core's batch, its 4 heads (256 dims), fp16.
            a2a_in = [dram.tile([NCORES, 64, 4 * HD], f16, name=f"a2a_in{g}")
                      for g in range(NG)]
            a2a_out = [dram.tile([NCORES, 64, 4 * HD], f16, name=f"a2a_out{g}")
                       for g in range(NG)]

            # score/softmax operand pools (live through all attention units)
            qk = ctx.enter_context(tc.tile_pool(name="qk", bufs=1))
            sb = ctx.enter_context(tc.tile_pool(name="sb", bufs=3))
            small = ctx.enter_context(tc.tile_pool(name="small", bufs=4))
            psn = ctx.enter_context(tc.tile_pool(name="psn", bufs=2, space="PSUM"))
            pss = ctx.enter_context(tc.tile_pool(name="pss", bufs=2, space="PSUM"))
            pso = ctx.enter_context(tc.tile_pool(name="pso", bufs=1, space="PSUM"))
            psm = ctx.enter_context(tc.tile_pool(name="psm", bufs=2, space="PSUM"))

            til_q, til_k = {}, {}
            for h in range(HPC):
                til_q[h] = qk.tile([65, T], f16, name=f"til_q{h}", tag="tq", bufs=HPC)
                til_k[h] = qk.tile([65, T], f16, name=f"til_k{h}", tag="tk", bufs=HPC)
                nc.gpsimd.memset(til_k[h][64:65, :], 1.0)
            vn = []
            for kc in range(NKC):
                v = qk.tile([128, HPC, 65], f16, name=f"vn{kc}", tag="vn", bufs=NKC)
                nc.gpsimd.memset(v[:, :, 64:65], 1.0)
                vn.append(v)

            # ---------------- QKV projections (own scope: frees w/x tiles) --
            qctx = contextlib.ExitStack()
            with qctx:
                wpool = qctx.enter_context(tc.tile_pool(name="wpool", bufs=1))
                xgp = qctx.enter_context(tc.tile_pool(name="xgp", bufs=4))

                w_sb, xgs = {}, []

                def load_w(w):
                    w_sb[w] = wpool.tile([128, KCH, 4 * HD], f16, name=f"sb_{w}")
                    nc.sync.dma_start(
                        w_sb[w][:], ap[w].rearrange("(a p) c -> p a c", p=128))

                def load_xg(g):
                    gsl = slice(512 * g, 512 * (g + 1))
                    xg = xgp.tile([128, KCH, 512], f16, name=f"xg{g}", tag="xg", bufs=4)
                    nc.sync.dma_start(
                        xg[:], ap["xT"].rearrange("(a p) t -> p a t", p=128)[:, :, gsl])
                    xgs.append(xg)

                load_w("wk")
                load_xg(0)
                load_w("wq")
                load_w("wv")
                for g in range(1, NG):
                    load_xg(g)

                def proj_pass(name, til, g):
                    gsl = slice(512 * g, 512 * (g + 1))
                    for hp in range(2):  # head pairs
                        p = pss.tile([128, 512], f32, tag="st", name="pqk")
                        for a in range(KCH):
                            nc.tensor.matmul(
                                p[:], w_sb[name][:, a, 128 * hp:128 * (hp + 1)],
                                xgs[g][:, a, :], start=(a == 0), stop=(a == KCH - 1))
                        for hl in range(2):
                            h = 2 * hp + hl
                            rows = slice(64 * hl, 64 * (hl + 1))
                            nc.scalar.activation(til[h][0:64, gsl], p[rows, :], AF.Copy)

                for g in range(NG):
                    proj_pass("wk", til_k, g)
                for g in range(NG):
                    proj_pass("wq", til_q, g)
                    for tt in range(4):  # V natural per token tile
                        kc = 4 * g + tt
                        p = pss.tile([128, 4 * HD], f32, tag="st", name="pv")
                        for a in range(KCH):
                            nc.tensor.matmul(
                                p[:], xgs[g][:, a, 128 * tt:128 * (tt + 1)],
                                w_sb["wv"][:, a, :], start=(a == 0), stop=(a == KCH - 1))
                        nc.scalar.activation(
                            vn[kc][:, :, 0:64],
                            p[:].rearrange("p (h d) -> p h d", h=HPC), AF.Copy)

            # ---------------- FFN pools (reuse qkv space; weights stream
            # during attention) ----------------
            w1p = ctx.enter_context(tc.tile_pool(name="w1p", bufs=1))
            w2p = ctx.enter_context(tc.tile_pool(name="w2p", bufs=1))
            fsb = ctx.enter_context(tc.tile_pool(name="fsb", bufs=2))
            o1p = ctx.enter_context(tc.tile_pool(name="o1p", bufs=1))
            fsm = ctx.enter_context(tc.tile_pool(name="fsm", bufs=4))
            psf = ctx.enter_context(tc.tile_pool(name="psf", bufs=1, space="PSUM"))

            w1t = w1p.tile([128, KCH, FF], f16, name="w1t")
            nc.sync.dma_start(
                w1t[:], ap["w1"].rearrange("(a p) f -> p a f", p=128))
            w2t = w2p.tile([128, FF // 128, H], f16, name="w2t")
            nc.sync.dma_start(
                w2t[:], ap["w2"].rearrange("(a p) o -> p a o", p=128))

            lnbc = {}
            for i, nm in enumerate(("g1", "b1", "g2", "b2")):
                lnbc[nm] = o1p.tile([128, H], f16, name=f"ln_{nm}", tag="lnbc", bufs=4)
                nc.sync.dma_start(
                    lnbc[nm][:], ap["lnw"][i, :].partition_broadcast(128))


            # ---------------- attention stages ----------------
            def stage_a1_gen(h, g):
                # natural-S matmuls + DVE max reduces, yielded one half-step
                # at a time so stage_b emission can interleave them (avoids
                # rate-limiting PE on the 2-slot sn ring)
                mstage = small.tile([128, 4], f16, tag="mstage", name="mstage",
                                    bufs=3)

                def gen():
                    for qt in range(4):
                        qsl = slice(512 * g + 128 * qt, 512 * g + 128 * (qt + 1))
                        negmax = []
                        for half in range(4):
                            sn = psn.tile([128, 512], f32, name="sn")
                            ks = slice(512 * half, 512 * (half + 1))
                            nc.tensor.matmul(
                                sn[:], til_q[h][0:64, qsl], til_k[h][0:64, ks],
                                start=True, stop=True)
                            nm = small.tile([128, 1], f32, tag="nm", name="nm",
                                            bufs=8)
                            nc.vector.tensor_reduce(
                                nm[:], sn[:], axis=mybir.AxisListType.X,
                                op=ALU.max, negate=True)
                            negmax.append(nm)
                            yield
                        nc.vector.tensor_tensor(
                            negmax[0][:], negmax[0][:], negmax[1][:], ALU.min)
                        nc.vector.tensor_tensor(
                            negmax[2][:], negmax[2][:], negmax[3][:], ALU.min)
                        nc.vector.tensor_tensor(
                            mstage[:, qt:qt + 1], negmax[0][:], negmax[2][:],
                            ALU.min)
                    while True:
                        yield

                return mstage, gen()

            def stage_a2(h, g, mstage):
                # emitted a period later so the PE transpose never waits on DVE
                for qt in range(4):
                    qsl = slice(512 * g + 128 * qt, 512 * g + 128 * (qt + 1))
                    mt = psm.tile([1, 128], f16, tag="mt", name="mt")
                    nc.tensor.transpose(mt[:], mstage[:, qt:qt + 1], identh[:])
                    nc.gpsimd.tensor_copy(til_q[h][64:65, qsl], mt[:])

            def stage_b(h, g, inter=None):
                gsl = slice(512 * g, 512 * (g + 1))
                o_acc = pso.tile([65, 512], f32, name="o_acc")
                pts = {}
                PVLAG = 2

                def pv(kc):
                    nc.tensor.matmul(o_acc[:], vn[kc][:, h, :], pts.pop(kc)[:],
                                     start=(kc == 0), stop=(kc == NKC - 1))

                for kc in range(NKC):
                    if inter is not None:
                        next(inter)
                    ksl = slice(128 * kc, 128 * (kc + 1))
                    st = pss.tile([128, 512], f32, tag="st", name="st")
                    nc.tensor.matmul(st[:], til_k[h][0:65, ksl],
                                     til_q[h][0:65, gsl], start=True, stop=True)
                    pt = sb.tile([128, 512], f16, tag="pt", name="pt", bufs=3)
                    nc.scalar.activation(pt[:], st[:], AF.Exp)
                    pts[kc] = pt
                    if kc >= PVLAG:
                        pv(kc - PVLAG)
                for kc in range(NKC - PVLAG, NKC):
                    pv(kc)
                ot = sb.tile([65, 512], f16, tag="ot", name="ot", bufs=2)
                nc.gpsimd.tensor_copy(ot[:], o_acc[:])
                # transpose to natural, scale by 1/denom, ship to a2a chunk g:
                # token rows 0:64 -> slot 2*tt, rows 64:128 -> slot 2*tt+1
                for tt in range(4):
                    op_ = psm.tile([128, 65], f16, tag="mt", name="opt")
                    nc.tensor.transpose(
                        op_[:], ot[0:65, 128 * tt:128 * (tt + 1)],
                        identh[0:65, 0:65])
                    rc = small.tile([128, 1], f32, tag="rc", name="rc")
                    nc.vector.reciprocal(rc[:], op_[:, 64:65])
                    ob = sb.tile([128, HD], f16, tag="ob", name="ob", bufs=2)
                    nc.gpsimd.tensor_scalar_mul(ob[:], op_[:, 0:64], rc[:])
                    nc.sync.dma_start(
                        a2a_in[g][2 * tt:2 * tt + 2, :, 64 * h:64 * (h + 1)],
                        ob[:].rearrange("(s p) d -> s p d", s=2))

            # ---------------- per-chunk FFN chain ----------------
            def layer_norm_to(dst, src, g_bc, b_bc, work):
                """dst = gamma*(src-mean)/(std_unbiased+EPS)+beta, [128,H]."""
                stats = fsm.tile([128, 2, 6], f32, tag="stats", name="stats")
                for hf in range(2):
                    nc.vector.bn_stats(stats[:, hf, :],
                                       src[:, 512 * hf:512 * (hf + 1)])
                mv = fsm.tile([128, 2], f32, tag="mv", name="mv")
                nc.vector.bn_aggr(mv[:], stats[:])
                # 1/std_unbiased == exp(-0.5*(ln(var) + ln(H/(H-1)))); Ln and
                # Exp share an act table set, so no table reload vs the
                # attention exps (the +1e-6-on-std term is 3e-8 relative).
                lv = fsm.tile([128, 1], f32, tag="sd", name="lv")
                nc.scalar.activation(lv[:], mv[:, 1:2], AF.Ln)
                rs = fsm.tile([128, 1], f32, tag="rs", name="rs")
                nc.scalar.activation(rs[:], lv[:], AF.Exp, scale=-0.5)
                nc.vector.tensor_scalar(out=work[:], in0=src[:],
                                        scalar1=mv[:, 0:1], scalar2=rs[:],
                                        op0=ALU.subtract, op1=ALU.mult)
                nc.vector.tensor_mul(work[:], work[:], g_bc[:])
                nc.vector.tensor_add(dst[:], work[:], b_bc[:])

            out1s = {}

            def ffn_front(g):
                if sim_single:
                    nc.sync.dma_start(a2a_out[g][:], a2a_in[g][:])
                else:
                    nc.gpsimd.collective_compute(
                        "AllToAll", ALU.bypass,
                        replica_groups=[list(range(NCORES))],
                        ins=[a2a_in[g].opt()], outs=[a2a_out[g].opt()])
                # assemble attention-out tile: rows 0:64 batch-0 block,
                # rows 64:128 batch-1 block; src r covers dims 256*(r%4)
                at = fsb.tile([128, H], f16, tag="ta", name="at", bufs=1)
                for r in range(NCORES):
                    rh, cs = (r // 4) * 64, 256 * (r % 4)
                    nc.sync.dma_start(at[rh:rh + 64, cs:cs + 256],
                                      a2a_out[g][r, :, :])
                xr = fsm.tile([128, H], f16, tag="xr", name="xr", bufs=1)
                nc.sync.dma_start(xr[:], ap["xres"][128 * g:128 * (g + 1), :])
                nc.vector.tensor_add(at[:], at[:], xr[:])
                out1 = fsb.tile([128, H], f16, tag="o1", name="out1")
                wk_ = fsb.tile([128, H], f16, tag="td", name="wk_", bufs=1)
                layer_norm_to(out1, at, lnbc["g1"], lnbc["b1"], wk_)
                out1s[g] = out1

            def ffn_back(g):
                out1 = out1s.pop(g)
                # transpose LN1 out for the H-contraction of FFN1
                out1T = fsb.tile([128, KCH, 128], f16, tag="o1T", name="out1T",
                                 bufs=1)
                for a in range(KCH):
                    tp = psm.tile([128, 128], f16, tag="mt", name="tp")
                    nc.tensor.transpose(
                        tp[:], out1[:, 128 * a:128 * (a + 1)], identh[:])
                    nc.gpsimd.tensor_copy(out1T[:, a, :], tp[:])
                # FFN1: ht[f, t] = relu(W1^T x out1T) for this 128-token tile
                ht = fsb.tile([128, FF // 128, 128], f16, tag="ht", name="ht",
                              bufs=1)
                for ft in range(FF // 128):
                    hp_ = psf.tile([128, 128], f32, tag="hp", name="hp", bufs=1)
                    for a in range(KCH):
                        nc.tensor.matmul(
                            hp_[:], w1t[:, a, 128 * ft:128 * (ft + 1)],
                            out1T[:, a, :], start=(a == 0), stop=(a == KCH - 1))
                    nc.gpsimd.tensor_scalar_max(ht[:, ft, :], hp_[:], 0.0)
                # FFN2 + fused LN2 tail
                h2 = fsb.tile([128, H], f16, tag="tc", name="h2", bufs=1)
                for oc in range(2):
                    acc = pss.tile([128, 512], f32, tag="st", name="o2acc")
                    for ft in range(FF // 128):
                        nc.tensor.matmul(
                            acc[:], ht[:, ft, :],
                            w2t[:, ft, 512 * oc:512 * (oc + 1)],
                            start=(ft == 0), stop=(ft == FF // 128 - 1))
                    nc.gpsimd.tensor_copy(h2[:, 512 * oc:512 * (oc + 1)], acc[:])
                nc.vector.tensor_add(h2[:], h2[:], out1[:])
                fin = fsb.tile([128, H], f16, tag="tb2", name="fin", bufs=1)
                wk2 = fsb.tile([128, H], f16, tag="td", name="wk2", bufs=1)
                layer_norm_to(fin, h2, lnbc["g2"], lnbc["b2"], wk2)
                nc.sync.dma_start(out_ap[128 * g:128 * (g + 1), :], fin[:])

            # ---------------- main loop: g-outer units; a1(i+2) interleaved
            # into b(i); a2a+LN1 for group g emitted right after its last
            # head, the PE-heavy FFN back-half 3 units later ----------------
            units = [(h, g) for g in range(NG) for h in range(HPC)]
            LOOKAHEAD = 2
            pend = {}
            for k in range(LOOKAHEAD):
                ms, gen = stage_a1_gen(*units[k])
                for _ in range(16):
                    next(gen)
                stage_a2(*units[k], ms)
            for i, (h, g) in enumerate(units):
                j = i + LOOKAHEAD
                if j < len(units):
                    pend[j] = stage_a1_gen(*units[j])
                stage_b(h, g, pend[j][1] if j in pend else None)
                if j < len(units):
                    stage_a2(*units[j], pend.pop(j)[0])
                if i % HPC == 3:
                    ffn_front(i // HPC)
                if i % HPC == 2 and i // HPC >= 1:
                    ffn_back(i // HPC - 1)
            ffn_back(NG - 1)

    nc.compile()
    if not sim_single:
        nc.m = get_hw_module(nc.m)
    return nc


_NC_CACHE = {}


def _get_program():
    if "nc" not in _NC_CACHE:
        _NC_CACHE["nc"] = _build_program()
    return _NC_CACHE["nc"]


def _prep_inputs(x, Wqkv, bqkv, W1, b1, W2, b2, gamma1, beta1, gamma2, beta2):
    """Host-side slicing/folding into per-core in_maps."""
    x = np.asarray(x, np.float32)
    Wqkv = np.asarray(Wqkv, np.float32)
    bqkv = np.asarray(bqkv, np.float32)
    d = np.arange(HD)
    hh = np.arange(NH)
    # qkv reshape in reference: [B,T,HD,3,NH] -> col = d*48 + k*16 + h
    cols = d[:, None, None] * (3 * NH) + np.arange(3)[None, :, None] * NH \
        + hh[None, None, :]
    Wq = Wqkv[:, cols[:, 0, :]] * (bqkv[cols[:, 0, :]] / np.sqrt(H))[None]
    Wk = Wqkv[:, cols[:, 1, :]] * bqkv[cols[:, 1, :]][None]
    Wv = Wqkv[:, cols[:, 2, :]] * bqkv[cols[:, 2, :]][None]
    # -> [H, HD, NH]; per-core head-major layout [H, 4*HD] (head-local major)
    Wq = np.transpose(Wq, (0, 2, 1)).astype(np.float16)  # [H, NH, HD]
    Wk = np.transpose(Wk, (0, 2, 1)).astype(np.float16)
    Wv = np.transpose(Wv, (0, 2, 1)).astype(np.float16)
    W1e = (np.asarray(W1, np.float32) * np.asarray(b1, np.float32)[None]) \
        .astype(np.float16)
    W2e = (np.asarray(W2, np.float32) * np.asarray(b2, np.float32)[None]) \
        .astype(np.float16)
    # rs is computed as 1/std_biased (exp(-0.5*ln(var))); fold the unbiased
    # correction sqrt((H-1)/H) into gamma
    gc = np.sqrt((H - 1.0) / H)
    lnw = np.stack([gamma1 * gc, beta1, gamma2 * gc, beta2]).astype(np.float16)
    xT = [np.ascontiguousarray(x[b].T).astype(np.float16) for b in range(B)]
    in_maps = []
    for c in range(NCORES):
        b, grp = c // 4, c % 4
        heads = slice(4 * grp, 4 * grp + 4)
        # FFN tokens of core c: per group g, batch b': x[b', 512g+64c : +64]
        xres = np.empty((TOK, H), np.float16)
        for g in range(NG):
            t0 = 512 * g + 64 * c
            xres[128 * g:128 * g + 64] = x[0, t0:t0 + 64]
            xres[128 * g + 64:128 * g + 128] = x[1, t0:t0 + 64]
        in_maps.append({
            "xT": xT[b],
            "xres": xres,
            "wq": np.ascontiguousarray(Wq[:, heads, :].reshape(H, 4 * HD)),
            "wk": np.ascontiguousarray(Wk[:, heads, :].reshape(H, 4 * HD)),
            "wv": np.ascontiguousarray(Wv[:, heads, :].reshape(H, 4 * HD)),
            "w1": W1e, "w2": W2e, "lnw": lnw,
        })
    return in_maps


def kernel(x, Wqkv, bqkv, W1, b1, W2, b2, gamma1, beta1, gamma2, beta2,
           _trace=False):
    nc = _get_program()
    in_maps = _prep_inputs(x, Wqkv, bqkv, W1, b1, W2, b2,
                           gamma1, beta1, gamma2, beta2)
    res = run_bass_kernel_spmd(nc, in_maps, core_ids=list(range(NCORES)),
                               trace=_trace)
    out = np.empty((B, T, H), np.float32)
    for c in range(NCORES):
        oc = res.results[c]["out"]  # [512, H]: (g, batch-half, 64)
        for g in range(NG):
            t0 = 512 * g + 64 * c
            out[0, t0:t0 + 64] = oc[128 * g:128 * g + 64]
            out[1, t0:t0 + 64] = oc[128 * g + 64:128 * g + 128]
    if _trace:
        kernel.last_results = res
    return out


# revision 34
# speedup vs baseline: 1.1357x; 1.0265x over previous
"""TRN2 Bass kernel for nn_DecoderLayer_47175920779446.

Full decoder layer: qkv (mul-bias) -> 16-head attention -> +res -> LN ->
FFN(relu, mul-bias) -> +res -> LN, on x[2, 2048, 1024] fp32.

Sharding (8 cores): attention is sharded by (batch, 4 heads): core c handles
batch c//4, heads 4*(c%4)..4*(c%4)+3 over all 2048 tokens of its batch.
FFN/LN are sharded by strided 64-token blocks: core c owns blocks
{t//64 == 8*m + c} of BOTH batches (256+256 tokens). Attention runs
query-group-outer; after each group g finishes (4 heads), a small fp16
AllToAll chunk (8 x [8, 64, 256]) reshards that group's attention output,
and the full FFN chain for that 128-token tile (LN1 -> FFN1 -> FFN2 -> LN2)
runs interleaved under the next attention group's matmuls.

Precision: scores run in fp16 (11-bit mantissa, matching the fp16 projection
noise floor ~2e-4*sigma): S = q16*k16 + m_hat (K=65 with a fused bias row).
V/P/FFN run fp16; LN stats and residual sums run fp32 where it matters.
"""
import contextlib
import numpy as np

import concourse.bass as bass
import concourse.tile as tile
from concourse import bacc, mybir
from concourse.bass_utils import run_bass_kernel_spmd
from concourse.bass_interp import get_hw_module
from concourse.masks import make_identity

H, NH, HD, FF = 1024, 16, 64, 4096
B, T = 2, 2048
EPS = 1e-6
NCORES = 8
HPC = NH // 4          # 4 heads per core
TOK = (B * T) // NCORES  # 512 tokens per core through the FFN
NKC = T // 128         # 16 key chunks
NG = T // 512          # 4 query groups
KCH = H // 128         # 8 contraction chunks for qkv
f32, f32r, bf16 = mybir.dt.float32, mybir.dt.float32r, mybir.dt.bfloat16
f16 = mybir.dt.float16
AF = mybir.ActivationFunctionType
ALU = mybir.AluOpType


def _build_program(sim_single=False):
    nc = bacc.Bacc("TRN2", target_bir_lowering=False, debug=False,
                   num_devices=1 if sim_single else NCORES)
    ap = {}
    ap["xT"] = nc.dram_tensor("xT", [H, T], f16, kind="ExternalInput").ap()
    ap["xres"] = nc.dram_tensor("xres", [TOK, H], f16, kind="ExternalInput").ap()
    for w in ("wq", "wk", "wv"):
        ap[w] = nc.dram_tensor(w, [H, 4 * HD], f16, kind="ExternalInput").ap()
    ap["w1"] = nc.dram_tensor("w1", [H, FF], f16, kind="ExternalInput").ap()
    ap["w2"] = nc.dram_tensor("w2", [FF, H], f16, kind="ExternalInput").ap()
    ap["lnw"] = nc.dram_tensor("lnw", [4, H], f16, kind="ExternalInput").ap()
    out_ap = nc.dram_tensor("out", [TOK, H], f16, kind="ExternalOutput").ap()

    with tile.TileContext(nc) as tc:
        ctx = contextlib.ExitStack()
        with ctx:
            const = ctx.enter_context(tc.tile_pool(name="const", bufs=1))
            dram = ctx.enter_context(tc.tile_pool(name="dram", bufs=1, space="DRAM"))

            identh = const.tile([128, 128], f16)
            make_identity(nc, identh[:])

            # per-group a2a chunks: slot r holds 64-token block (t//64 == 8g+r)
            # of this core's batch, its 4 heads (256 dims), fp16.
            a2a_in = [dram.tile([NCORES, 64, 4 * HD], f16, name=f"a2a_in{g}")
                      for g in range(NG)]
            a2a_out = [dram.tile([NCORES, 64, 4 * HD], f16, name=f"a2a_out{g}")
                       for g in range(NG)]

            # score/softmax operand pools (live through all attention units)
            qk = ctx.enter_context(tc.tile_pool(name="qk", bufs=1))
            sb = ctx.enter_context(tc.tile_pool(name="sb", bufs=3))
            small = ctx.enter_context(tc.tile_pool(name="small", bufs=4))
            psn = ctx.enter_context(tc.tile_pool(name="psn", bufs=2, space="PSUM"))
            pss = ctx.enter_context(tc.tile_pool(name="pss", bufs=2, space="PSUM"))
            pso = ctx.enter_context(tc.tile_pool(name="pso", bufs=1, space="PSUM"))
            psm = ctx.enter_context(tc.tile_pool(name="psm", bufs=2, space="PSUM"))

            til_q, til_k = {}, {}
            for h in range(HPC):
                til_q[h] = qk.tile([65, T], f16, name=f"til_q{h}", tag="tq", bufs=HPC)
                til_k[h] = qk.tile([65, T], f16, name=f"til_k{h}", tag="tk", bufs=HPC)
                nc.gpsimd.memset(til_k[h][64:65, :], 1.0)
            vn = []
            for kc in range(NKC):
                v = qk.tile([128, HPC, 65], f16, name=f"vn{kc}", tag="vn", bufs=NKC)
                nc.gpsimd.memset(v[:, :, 64:65], 1.0)
                vn.append(v)

            # ---------------- QKV projections (own scope: frees w/x tiles) --
            qctx = contextlib.ExitStack()
            with qctx:
                wpool = qctx.enter_context(tc.tile_pool(name="wpool", bufs=1))
                xgp = qctx.enter_context(tc.tile_pool(name="xgp", bufs=4))

                w_sb, xgs = {}, []

                def load_w(w):
                    w_sb[w] = wpool.tile([128, KCH, 4 * HD], f16, name=f"sb_{w}")
                    nc.sync.dma_start(
                        w_sb[w][:], ap[w].rearrange("(a p) c -> p a c", p=128))

                def load_xg(g):
                    gsl = slice(512 * g, 512 * (g + 1))
                    xg = xgp.tile([128, KCH, 512], f16, name=f"xg{g}", tag="xg", bufs=4)
                    nc.sync.dma_start(
                        xg[:], ap["xT"].rearrange("(a p) t -> p a t", p=128)[:, :, gsl])
                    xgs.append(xg)

                load_w("wk")
                load_xg(0)
                load_w("wq")
                load_w("wv")
                for g in range(1, NG):
                    load_xg(g)

                def proj_pass(name, til, g):
                    gsl = slice(512 * g, 512 * (g + 1))
                    for hp in range(2):  # head pairs
                        p = pss.tile([128, 512], f32, tag="st", name="pqk")
                        for a in range(KCH):
                            nc.tensor.matmul(
                                p[:], w_sb[name][:, a, 128 * hp:128 * (hp + 1)],
                                xgs[g][:, a, :], start=(a == 0), stop=(a == KCH - 1))
                        for hl in range(2):
                            h = 2 * hp + hl
                            rows = slice(64 * hl, 64 * (hl + 1))
                            nc.scalar.activation(til[h][0:64, gsl], p[rows, :], AF.Copy)

                for g in range(NG):
                    proj_pass("wk", til_k, g)
                for g in range(NG):
                    proj_pass("wq", til_q, g)
                    for tt in range(4):  # V natural per token tile
                        kc = 4 * g + tt
                        p = pss.tile([128, 4 * HD], f32, tag="st", name="pv")
                        for a in range(KCH):
                            nc.tensor.matmul(
                                p[:], xgs[g][:, a, 128 * tt:128 * (tt + 1)],
                                w_sb["wv"][:, a, :], start=(a == 0), stop=(a == KCH - 1))
                        nc.scalar.activation(
                            vn[kc][:, :, 0:64],
                            p[:].rearrange("p (h d) -> p h d", h=HPC), AF.Copy)

            # ---------------- FFN pools (reuse qkv space; weights stream
            # during attention) ----------------
            w1p = ctx.enter_context(tc.tile_pool(name="w1p", bufs=1))
            w2p = ctx.enter_context(tc.tile_pool(name="w2p", bufs=1))
            fsb = ctx.enter_context(tc.tile_pool(name="fsb", bufs=2))
            o1p = ctx.enter_context(tc.tile_pool(name="o1p", bufs=1))
            fsm = ctx.enter_context(tc.tile_pool(name="fsm", bufs=4))
            psf = ctx.enter_context(tc.tile_pool(name="psf", bufs=1, space="PSUM"))

            w1t = w1p.tile([128, KCH, FF], f16, name="w1t")
            nc.sync.dma_start(
                w1t[:], ap["w1"].rearrange("(a p) f -> p a f", p=128))
            w2t = w2p.tile([128, FF // 128, H], f16, name="w2t")
            nc.sync.dma_start(
                w2t[:], ap["w2"].rearrange("(a p) o -> p a o", p=128))

            lnbc = {}
            for i, nm in enumerate(("g1", "b1", "g2", "b2")):
                lnbc[nm] = o1p.tile([128, H], f16, name=f"ln_{nm}", tag="lnbc", bufs=4)
                nc.sync.dma_start(
                    lnbc[nm][:], ap["lnw"][i, :].partition_broadcast(128))


            # ---------------- attention stages ----------------
            def stage_a1_gen(h, g):
                # natural-S matmuls + DVE max reduces, yielded one half-step
                # at a time so stage_b emission can interleave them (avoids
                # rate-limiting PE on the 2-slot sn ring)
                mstage = small.tile([128, 4], f16, tag="mstage", name="mstage",
                                    bufs=3)

                def gen():
                    for qt in range(4):
                        qsl = slice(512 * g + 128 * qt, 512 * g + 128 * (qt + 1))
                        negmax = []
                        for half in range(4):
                            sn = psn.tile([128, 512], f32, name="sn")
                            ks = slice(512 * half, 512 * (half + 1))
                            nc.tensor.matmul(
                                sn[:], til_q[h][0:64, qsl], til_k[h][0:64, ks],
                                start=True, stop=True)
                            nm = small.tile([128, 1], f32, tag="nm", name="nm",
                                            bufs=8)
                            nc.vector.tensor_reduce(
                                nm[:], sn[:], axis=mybir.AxisListType.X,
                                op=ALU.max, negate=True)
                            negmax.append(nm)
                            yield
                        nc.vector.tensor_tensor(
                            negmax[0][:], negmax[0][:], negmax[1][:], ALU.min)
                        nc.vector.tensor_tensor(
                            negmax[2][:], negmax[2][:], negmax[3][:], ALU.min)
                        nc.vector.tensor_tensor(
                            mstage[:, qt:qt + 1], negmax[0][:], negmax[2][:],
                            ALU.min)
                    while True:
                        yield

                return mstage, gen()

            def stage_a2(h, g, mstage):
                # emitted a period later so the PE transpose never waits on DVE
                for qt in range(4):
                    qsl = slice(512 * g + 128 * qt, 512 * g + 128 * (qt + 1))
                    mt = psm.tile([1, 128], f16, tag="mt", name="mt")
                    nc.tensor.transpose(mt[:], mstage[:, qt:qt + 1], identh[:])
                    nc.gpsimd.tensor_copy(til_q[h][64:65, qsl], mt[:])

            def stage_b(h, g, inter=None):
                gsl = slice(512 * g, 512 * (g + 1))
                o_acc = pso.tile([65, 512], f32, name="o_acc")
                pts = {}
                PVLAG = 2

                def pv(kc):
                    nc.tensor.matmul(o_acc[:], vn[kc][:, h, :], pts.pop(kc)[:],
                                     start=(kc == 0), stop=(kc == NKC - 1))

                for kc in range(NKC):
                    if inter is not None:
                        next(inter)
                    ksl = slice(128 * kc, 128 * (kc + 1))
                    st = pss.tile([128, 512], f32, tag="st", name="st")
                    nc.tensor.matmul(st[:], til_k[h][0:65, ksl],
                                     til_q[h][0:65, gsl], start=True, stop=True)
                    pt = sb.tile([128, 512], f16, tag="pt", name="pt", bufs=3)
                    nc.scalar.activation(pt[:], st[:], AF.Exp)
                    pts[kc] = pt
                    if kc >= PVLAG:
                        pv(kc - PVLAG)
                for kc in range(NKC - PVLAG, NKC):
                    pv(kc)
                ot = sb.tile([65, 512], f16, tag="ot", name="ot", bufs=2)
                nc.gpsimd.tensor_copy(ot[:], o_acc[:])
                # transpose to natural, scale by 1/denom, ship to a2a chunk g:
                # token rows 0:64 -> slot 2*tt, rows 64:128 -> slot 2*tt+1
                for tt in range(4):
                    op_ = psm.tile([128, 65], f16, tag="mt", name="opt")
                    nc.tensor.transpose(
                        op_[:], ot[0:65, 128 * tt:128 * (tt + 1)],
                        identh[0:65, 0:65])
                    rc = small.tile([128, 1], f32, tag="rc", name="rc")
                    nc.vector.reciprocal(rc[:], op_[:, 64:65])
                    ob = sb.tile([128, HD], f16, tag="ob", name="ob", bufs=2)
                    nc.gpsimd.tensor_scalar_mul(ob[:], op_[:, 0:64], rc[:])
                    nc.sync.dma_start(
                        a2a_in[g][2 * tt:2 * tt + 2, :, 64 * h:64 * (h + 1)],
                        ob[:].rearrange("(s p) d -> s p d", s=2))

            # ---------------- per-chunk FFN chain ----------------
            def layer_norm_to(dst, src, g_bc, b_bc, work):
                """dst = gamma*(src-mean)/(std_unbiased+EPS)+beta, [128,H]."""
                stats = fsm.tile([128, 2, 6], f32, tag="stats", name="stats")
                for hf in range(2):
                    nc.vector.bn_stats(stats[:, hf, :],
                                       src[:, 512 * hf:512 * (hf + 1)])
                mv = fsm.tile([128, 2], f32, tag="mv", name="mv")
                nc.vector.bn_aggr(mv[:], stats[:])
                # 1/std via DVE pow(var, -0.5): keeps the ACT engine on the
                # Exp table set all kernel (the unbiased-var factor is folded
                # into gamma host-side; the +1e-6-on-std term is 3e-8 rel).
                rs = fsm.tile([128, 1], f32, tag="rs", name="rs")
                nc.vector.tensor_scalar(out=rs[:], in0=mv[:, 1:2], scalar1=-0.5,
                                        scalar2=None, op0=ALU.pow)
                nc.vector.tensor_scalar(out=work[:], in0=src[:],
                                        scalar1=mv[:, 0:1], scalar2=rs[:],
                                        op0=ALU.subtract, op1=ALU.mult)
                nc.vector.tensor_mul(work[:], work[:], g_bc[:])
                nc.vector.tensor_add(dst[:], work[:], b_bc[:])

            out1s = {}

            def ffn_front(g):
                if sim_single:
                    nc.sync.dma_start(a2a_out[g][:], a2a_in[g][:])
                else:
                    nc.gpsimd.collective_compute(
                        "AllToAll", ALU.bypass,
                        replica_groups=[list(range(NCORES))],
                        ins=[a2a_in[g].opt()], outs=[a2a_out[g].opt()])
                # assemble attention-out tile: rows 0:64 batch-0 block,
                # rows 64:128 batch-1 block; src r covers dims 256*(r%4)
                at = fsb.tile([128, H], f16, tag="ta", name="at", bufs=1)
                for r in range(NCORES):
                    rh, cs = (r // 4) * 64, 256 * (r % 4)
                    nc.sync.dma_start(at[rh:rh + 64, cs:cs + 256],
                                      a2a_out[g][r, :, :])
                xr = fsm.tile([128, H], f16, tag="xr", name="xr", bufs=1)
                nc.sync.dma_start(xr[:], ap["xres"][128 * g:128 * (g + 1), :])
                nc.vector.tensor_add(at[:], at[:], xr[:])
                out1 = fsb.tile([128, H], f16, tag="o1", name="out1")
                wk_ = fsb.tile([128, H], f16, tag="td", name="wk_", bufs=1)
                layer_norm_to(out1, at, lnbc["g1"], lnbc["b1"], wk_)
                out1s[g] = out1

            def ffn_back(g):
                out1 = out1s.pop(g)
                # transpose LN1 out for the H-contraction of FFN1
                out1T = fsb.tile([128, KCH, 128], f16, tag="o1T", name="out1T",
                                 bufs=1)
                for a in range(KCH):
                    tp = psm.tile([128, 128], f16, tag="mt", name="tp")
                    nc.tensor.transpose(
                        tp[:], out1[:, 128 * a:128 * (a + 1)], identh[:])
                    nc.gpsimd.tensor_copy(out1T[:, a, :], tp[:])
                # FFN1: ht[f, t] = relu(W1^T x out1T) for this 128-token tile
                ht = fsb.tile([128, FF // 128, 128], f16, tag="ht", name="ht",
                              bufs=1)
                for ft in range(FF // 128):
                    hp_ = psf.tile([128, 128], f32, tag="hp", name="hp", bufs=1)
                    for a in range(KCH):
                        nc.tensor.matmul(
                            hp_[:], w1t[:, a, 128 * ft:128 * (ft + 1)],
                            out1T[:, a, :], start=(a == 0), stop=(a == KCH - 1))
                    nc.gpsimd.tensor_scalar_max(ht[:, ft, :], hp_[:], 0.0)
                # FFN2 + fused LN2 tail
                h2 = fsb.tile([128, H], f16, tag="tc", name="h2", bufs=1)
                for oc in range(2):
                    acc = pss.tile([128, 512], f32, tag="st", name="o2acc")
                    for ft in range(FF // 128):
                        nc.tensor.matmul(
                            acc[:], ht[:, ft, :],
                            w2t[:, ft, 512 * oc:512 * (oc + 1)],
                            start=(ft == 0), stop=(ft == FF // 128 - 1))
                    nc.gpsimd.tensor_copy(h2[:, 512 * oc:512 * (oc + 1)], acc[:])
                nc.vector.tensor_add(h2[:], h2[:], out1[:])
                fin = fsb.tile([128, H], f16, tag="tb2", name="fin", bufs=1)
                wk2 = fsb.tile([128, H], f16, tag="td", name="wk2", bufs=1)
                layer_norm_to(fin, h2, lnbc["g2"], lnbc["b2"], wk2)
                nc.sync.dma_start(out_ap[128 * g:128 * (g + 1), :], fin[:])

            # ---------------- main loop: g-outer units; a1(i+2) interleaved
            # into b(i); a2a+LN1 for group g emitted right after its last
            # head, the PE-heavy FFN back-half 3 units later ----------------
            units = [(h, g) for g in range(NG) for h in range(HPC)]
            LOOKAHEAD = 2
            pend = {}
            for k in range(LOOKAHEAD):
                ms, gen = stage_a1_gen(*units[k])
                for _ in range(16):
                    next(gen)
                stage_a2(*units[k], ms)
            for i, (h, g) in enumerate(units):
                j = i + LOOKAHEAD
                if j < len(units):
                    pend[j] = stage_a1_gen(*units[j])
                stage_b(h, g, pend[j][1] if j in pend else None)
                if j < len(units):
                    stage_a2(*units[j], pend.pop(j)[0])
                if i % HPC == 3:
                    ffn_front(i // HPC)
                if i % HPC == 2 and i // HPC >= 1:
                    ffn_back(i // HPC - 1)
            ffn_back(NG - 1)

    nc.compile()
    if not sim_single:
        nc.m = get_hw_module(nc.m)
    return nc


_NC_CACHE = {}


def _get_program():
    if "nc" not in _NC_CACHE:
        _NC_CACHE["nc"] = _build_program()
    return _NC_CACHE["nc"]


def _prep_inputs(x, Wqkv, bqkv, W1, b1, W2, b2, gamma1, beta1, gamma2, beta2):
    """Host-side slicing/folding into per-core in_maps."""
    x = np.asarray(x, np.float32)
    Wqkv = np.asarray(Wqkv, np.float32)
    bqkv = np.asarray(bqkv, np.float32)
    d = np.arange(HD)
    hh = np.arange(NH)
    # qkv reshape in reference: [B,T,HD,3,NH] -> col = d*48 + k*16 + h
    cols = d[:, None, None] * (3 * NH) + np.arange(3)[None, :, None] * NH \
        + hh[None, None, :]
    Wq = Wqkv[:, cols[:, 0, :]] * (bqkv[cols[:, 0, :]] / np.sqrt(H))[None]
    Wk = Wqkv[:, cols[:, 1, :]] * bqkv[cols[:, 1, :]][None]
    Wv = Wqkv[:, cols[:, 2, :]] * bqkv[cols[:, 2, :]][None]
    # -> [H, HD, NH]; per-core head-major layout [H, 4*HD] (head-local major)
    Wq = np.transpose(Wq, (0, 2, 1)).astype(np.float16)  # [H, NH, HD]
    Wk = np.transpose(Wk, (0, 2, 1)).astype(np.float16)
    Wv = np.transpose(Wv, (0, 2, 1)).astype(np.float16)
    W1e = (np.asarray(W1, np.float32) * np.asarray(b1, np.float32)[None]) \
        .astype(np.float16)
    W2e = (np.asarray(W2, np.float32) * np.asarray(b2, np.float32)[None]) \
        .astype(np.float16)
    # rs is computed as 1/std_biased (exp(-0.5*ln(var))); fold the unbiased
    # correction sqrt((H-1)/H) into gamma
    gc = np.sqrt((H - 1.0) / H)
    lnw = np.stack([gamma1 * gc, beta1, gamma2 * gc, beta2]).astype(np.float16)
    xT = [np.ascontiguousarray(x[b].T).astype(np.float16) for b in range(B)]
    in_maps = []
    for c in range(NCORES):
        b, grp = c // 4, c % 4
        heads = slice(4 * grp, 4 * grp + 4)
        # FFN tokens of core c: per group g, batch b': x[b', 512g+64c : +64]
        xres = np.empty((TOK, H), np.float16)
        for g in range(NG):
            t0 = 512 * g + 64 * c
            xres[128 * g:128 * g + 64] = x[0, t0:t0 + 64]
            xres[128 * g + 64:128 * g + 128] = x[1, t0:t0 + 64]
        in_maps.append({
            "xT": xT[b],
            "xres": xres,
            "wq": np.ascontiguousarray(Wq[:, heads, :].reshape(H, 4 * HD)),
            "wk": np.ascontiguousarray(Wk[:, heads, :].reshape(H, 4 * HD)),
            "wv": np.ascontiguousarray(Wv[:, heads, :].reshape(H, 4 * HD)),
            "w1": W1e, "w2": W2e, "lnw": lnw,
        })
    return in_maps


def kernel(x, Wqkv, bqkv, W1, b1, W2, b2, gamma1, beta1, gamma2, beta2,
           _trace=False):
    nc = _get_program()
    in_maps = _prep_inputs(x, Wqkv, bqkv, W1, b1, W2, b2,
                           gamma1, beta1, gamma2, beta2)
    res = run_bass_kernel_spmd(nc, in_maps, core_ids=list(range(NCORES)),
                               trace=_trace)
    out = np.empty((B, T, H), np.float32)
    for c in range(NCORES):
        oc = res.results[c]["out"]  # [512, H]: (g, batch-half, 64)
        for g in range(NG):
            t0 = 512 * g + 64 * c
            out[0, t0:t0 + 64] = oc[128 * g:128 * g + 64]
            out[1, t0:t0 + 64] = oc[128 * g + 64:128 * g + 128]
    if _trace:
        kernel.last_results = res
    return out


# revision 36
# speedup vs baseline: 1.3135x; 1.1565x over previous
"""TRN2 Bass kernel for nn_DecoderLayer_47175920779446.

Full decoder layer: qkv (mul-bias) -> 16-head attention -> +res -> LN ->
FFN(relu, mul-bias) -> +res -> LN, on x[2, 2048, 1024] fp32.

Sharding (8 cores): attention is sharded by (batch, 4 heads): core c handles
batch c//4, heads 4*(c%4)..4*(c%4)+3 over all 2048 tokens of its batch.
FFN/LN are sharded by strided 64-token blocks: core c owns blocks
{t//64 == 8*m + c} of BOTH batches (256+256 tokens). Attention runs
query-group-outer; after each group g finishes (4 heads), a small fp16
AllToAll chunk (8 x [8, 64, 256]) reshards that group's attention output,
and the full FFN chain for that 128-token tile (LN1 -> FFN1 -> FFN2 -> LN2)
runs interleaved under the next attention group's matmuls.

Precision: scores run in fp16 (11-bit mantissa, matching the fp16 projection
noise floor ~2e-4*sigma): S = q16*k16 + m_hat (K=65 with a fused bias row).
V/P/FFN run fp16; LN stats and residual sums run fp32 where it matters.
"""
import contextlib
import numpy as np

import concourse.bass as bass
import concourse.tile as tile
from concourse import bacc, mybir
from concourse.bass_utils import run_bass_kernel_spmd
from concourse.bass_interp import get_hw_module
from concourse.masks import make_identity

H, NH, HD, FF = 1024, 16, 64, 4096
B, T = 2, 2048
EPS = 1e-6
NCORES = 8
HPC = NH // 4          # 4 heads per core
TOK = (B * T) // NCORES  # 512 tokens per core through the FFN
NKC = T // 128         # 16 key chunks
NG = T // 512          # 4 query groups
KCH = H // 128         # 8 contraction chunks for qkv
f32, f32r, bf16 = mybir.dt.float32, mybir.dt.float32r, mybir.dt.bfloat16
f16 = mybir.dt.float16
AF = mybir.ActivationFunctionType
ALU = mybir.AluOpType


def _build_program(sim_single=False):
    nc = bacc.Bacc("TRN2", target_bir_lowering=False, debug=False,
                   num_devices=1 if sim_single else NCORES)
    ap = {}
    ap["xT"] = nc.dram_tensor("xT", [H, T], f16, kind="ExternalInput").ap()
    ap["xres"] = nc.dram_tensor("xres", [TOK, H], f16, kind="ExternalInput").ap()
    for w in ("wq", "wk", "wv"):
        ap[w] = nc.dram_tensor(w, [H, 4 * HD], f16, kind="ExternalInput").ap()
    ap["w1"] = nc.dram_tensor("w1", [H, FF], f16, kind="ExternalInput").ap()
    ap["w2"] = nc.dram_tensor("w2", [FF, H], f16, kind="ExternalInput").ap()
    ap["lnw"] = nc.dram_tensor("lnw", [4, H], f16, kind="ExternalInput").ap()
    out_ap = nc.dram_tensor("out", [TOK, H], f16, kind="ExternalOutput").ap()

    with tile.TileContext(nc) as tc:
        ctx = contextlib.ExitStack()
        with ctx:
            const = ctx.enter_context(tc.tile_pool(name="const", bufs=1))
            dram = ctx.enter_context(tc.tile_pool(name="dram", bufs=1, space="DRAM"))

            identh = const.tile([128, 128], f16)
            make_identity(nc, identh[:])

            # per-group a2a chunks: slot r holds 64-token block (t//64 == 8g+r)
            # of this core's batch, its 4 heads (256 dims), fp16.
            a2a_in = [dram.tile([NCORES, 64, 4 * HD], f16, name=f"a2a_in{g}")
                      for g in range(NG)]
            a2a_out = [dram.tile([NCORES, 64, 4 * HD], f16, name=f"a2a_out{g}")
                       for g in range(NG)]

            # score/softmax operand pools (live through all attention units)
            qk = ctx.enter_context(tc.tile_pool(name="qk", bufs=1))
            sb = ctx.enter_context(tc.tile_pool(name="sb", bufs=3))
            small = ctx.enter_context(tc.tile_pool(name="small", bufs=4))
            psn = ctx.enter_context(tc.tile_pool(name="psn", bufs=2, space="PSUM"))
            pss = ctx.enter_context(tc.tile_pool(name="pss", bufs=2, space="PSUM"))
            pso = ctx.enter_context(tc.tile_pool(name="pso", bufs=1, space="PSUM"))
            psm = ctx.enter_context(tc.tile_pool(name="psm", bufs=2, space="PSUM"))

            til_q, til_k = {}, {}
            for h in range(HPC):
                til_q[h] = qk.tile([65, T], f16, name=f"til_q{h}", tag="tq", bufs=HPC)
                til_k[h] = qk.tile([65, T], f16, name=f"til_k{h}", tag="tk", bufs=HPC)
                nc.gpsimd.memset(til_k[h][64:65, :], 1.0)
            vn = []
            for kc in range(NKC):
                v = qk.tile([128, HPC, 65], f16, name=f"vn{kc}", tag="vn", bufs=NKC)
                nc.gpsimd.memset(v[:, :, 64:65], 1.0)
                vn.append(v)

            # ---------------- QKV projections (own scope: frees w/x tiles) --
            qctx = contextlib.ExitStack()
            with qctx:
                wpool = qctx.enter_context(tc.tile_pool(name="wpool", bufs=1))
                xgp = qctx.enter_context(tc.tile_pool(name="xgp", bufs=4))

                w_sb, xgs = {}, []

                def load_w(w):
                    w_sb[w] = wpool.tile([128, KCH, 4 * HD], f16, name=f"sb_{w}")
                    nc.sync.dma_start(
                        w_sb[w][:], ap[w].rearrange("(a p) c -> p a c", p=128))

                def load_xg(g):
                    gsl = slice(512 * g, 512 * (g + 1))
                    xg = xgp.tile([128, KCH, 512], f16, name=f"xg{g}", tag="xg", bufs=4)
                    nc.scalar.dma_start(
                        xg[:], ap["xT"].rearrange("(a p) t -> p a t", p=128)[:, :, gsl])
                    xgs.append(xg)

                load_w("wk")
                load_xg(0)
                load_w("wq")
                load_w("wv")
                for g in range(1, NG):
                    load_xg(g)

                def proj_pass(name, til, g):
                    gsl = slice(512 * g, 512 * (g + 1))
                    for hp in range(2):  # head pairs
                        p = pss.tile([128, 512], f32, tag="st", name="pqk")
                        for a in range(KCH):
                            nc.tensor.matmul(
                                p[:], w_sb[name][:, a, 128 * hp:128 * (hp + 1)],
                                xgs[g][:, a, :], start=(a == 0), stop=(a == KCH - 1))
                        for hl in range(2):
                            h = 2 * hp + hl
                            rows = slice(64 * hl, 64 * (hl + 1))
                            nc.scalar.activation(til[h][0:64, gsl], p[rows, :], AF.Copy)

                for g in range(NG):
                    proj_pass("wk", til_k, g)
                for g in range(NG):
                    proj_pass("wq", til_q, g)
                    for tt in range(4):  # V natural per token tile
                        kc = 4 * g + tt
                        p = pss.tile([128, 4 * HD], f32, tag="st", name="pv")
                        for a in range(KCH):
                            nc.tensor.matmul(
                                p[:], xgs[g][:, a, 128 * tt:128 * (tt + 1)],
                                w_sb["wv"][:, a, :], start=(a == 0), stop=(a == KCH - 1))
                        nc.scalar.activation(
                            vn[kc][:, :, 0:64],
                            p[:].rearrange("p (h d) -> p h d", h=HPC), AF.Copy)

            # ---------------- FFN pools (reuse qkv space; weights stream
            # during attention) ----------------
            w1p = ctx.enter_context(tc.tile_pool(name="w1p", bufs=1))
            w2p = ctx.enter_context(tc.tile_pool(name="w2p", bufs=1))
            fsb = ctx.enter_context(tc.tile_pool(name="fsb", bufs=2))
            o1p = ctx.enter_context(tc.tile_pool(name="o1p", bufs=1))
            fsm = ctx.enter_context(tc.tile_pool(name="fsm", bufs=4))
            psf = ctx.enter_context(tc.tile_pool(name="psf", bufs=1, space="PSUM"))

            w1t = w1p.tile([128, KCH, FF], f16, name="w1t")
            nc.sync.dma_start(
                w1t[:], ap["w1"].rearrange("(a p) f -> p a f", p=128))
            w2t = w2p.tile([128, FF // 128, H], f16, name="w2t")
            nc.sync.dma_start(
                w2t[:], ap["w2"].rearrange("(a p) o -> p a o", p=128))

            lnbc = {}
            for i, nm in enumerate(("g1", "b1", "g2", "b2")):
                lnbc[nm] = o1p.tile([128, H], f16, name=f"ln_{nm}", tag="lnbc", bufs=4)
                nc.sync.dma_start(
                    lnbc[nm][:], ap["lnw"][i, :].partition_broadcast(128))


            # ---------------- attention stages ----------------
            def stage_a1_gen(h, g):
                # natural-S matmuls + DVE max reduces, yielded one half-step
                # at a time so stage_b emission can interleave them (avoids
                # rate-limiting PE on the 2-slot sn ring)
                mstage = small.tile([128, 4], f16, tag="mstage", name="mstage",
                                    bufs=3)

                def gen():
                    for qt in range(4):
                        qsl = slice(512 * g + 128 * qt, 512 * g + 128 * (qt + 1))
                        negmax = []
                        for half in range(4):
                            sn = psn.tile([128, 512], f32, name="sn")
                            ks = slice(512 * half, 512 * (half + 1))
                            nc.tensor.matmul(
                                sn[:], til_q[h][0:64, qsl], til_k[h][0:64, ks],
                                start=True, stop=True)
                            nm = small.tile([128, 1], f32, tag="nm", name="nm",
                                            bufs=8)
                            nc.vector.tensor_reduce(
                                nm[:], sn[:], axis=mybir.AxisListType.X,
                                op=ALU.max, negate=True)
                            negmax.append(nm)
                            yield
                        nc.vector.tensor_tensor(
                            negmax[0][:], negmax[0][:], negmax[1][:], ALU.min)
                        nc.vector.tensor_tensor(
                            negmax[2][:], negmax[2][:], negmax[3][:], ALU.min)
                        nc.vector.tensor_tensor(
                            mstage[:, qt:qt + 1], negmax[0][:], negmax[2][:],
                            ALU.min)
                    while True:
                        yield

                return mstage, gen()

            def stage_a2(h, g, mstage):
                # emitted a period later so the PE transpose never waits on DVE
                for qt in range(4):
                    qsl = slice(512 * g + 128 * qt, 512 * g + 128 * (qt + 1))
                    mt = psm.tile([1, 128], f16, tag="mt", name="mt")
                    nc.tensor.transpose(mt[:], mstage[:, qt:qt + 1], identh[:])
                    nc.gpsimd.tensor_copy(til_q[h][64:65, qsl], mt[:])

            def stage_b(h, g, inter=None):
                gsl = slice(512 * g, 512 * (g + 1))
                o_acc = pso.tile([65, 512], f32, name="o_acc")
                pts = {}
                PVLAG = 2

                def pv(kc):
                    nc.tensor.matmul(o_acc[:], vn[kc][:, h, :], pts.pop(kc)[:],
                                     start=(kc == 0), stop=(kc == NKC - 1))

                for kc in range(NKC):
                    if inter is not None:
                        next(inter)
                    ksl = slice(128 * kc, 128 * (kc + 1))
                    st = pss.tile([128, 512], f32, tag="st", name="st")
                    nc.tensor.matmul(st[:], til_k[h][0:65, ksl],
                                     til_q[h][0:65, gsl], start=True, stop=True)
                    pt = sb.tile([128, 512], f16, tag="pt", name="pt", bufs=3)
                    nc.scalar.activation(pt[:], st[:], AF.Exp)
                    pts[kc] = pt
                    if kc >= PVLAG:
                        pv(kc - PVLAG)
                for kc in range(NKC - PVLAG, NKC):
                    pv(kc)
                ot = sb.tile([65, 512], f16, tag="ot", name="ot", bufs=2)
                nc.gpsimd.tensor_copy(ot[:], o_acc[:])
                # transpose to natural, scale by 1/denom, ship to a2a chunk g:
                # token rows 0:64 -> slot 2*tt, rows 64:128 -> slot 2*tt+1;
                # single batched DMA per unit (SP sequencer time is 565ns per
                # dma_start -- instruction count matters)
                obu = sb.tile([128, 4, HD], f16, tag="ob", name="obu", bufs=2)
                for tt in range(4):
                    op_ = psm.tile([128, 65], f16, tag="mt", name="opt")
                    nc.tensor.transpose(
                        op_[:], ot[0:65, 128 * tt:128 * (tt + 1)],
                        identh[0:65, 0:65])
                    rc = small.tile([128, 1], f32, tag="rc", name="rc")
                    nc.vector.reciprocal(rc[:], op_[:, 64:65])
                    nc.gpsimd.tensor_scalar_mul(obu[:, tt, :], op_[:, 0:64], rc[:])
                nc.sync.dma_start(
                    a2a_in[g][:, :, 64 * h:64 * (h + 1)]
                    .rearrange("(q s) p d -> (s p) q d", s=2),
                    obu[:])

            # ---------------- per-chunk FFN chain ----------------
            def layer_norm_to(dst, src, g_bc, b_bc, work):
                """dst = gamma*(src-mean)/(std_unbiased+EPS)+beta, [128,H]."""
                stats = fsm.tile([128, 2, 6], f32, tag="stats", name="stats")
                for hf in range(2):
                    nc.vector.bn_stats(stats[:, hf, :],
                                       src[:, 512 * hf:512 * (hf + 1)])
                mv = fsm.tile([128, 2], f32, tag="mv", name="mv")
                nc.vector.bn_aggr(mv[:], stats[:])
                # 1/std via DVE pow(var, -0.5): keeps the ACT engine on the
                # Exp table set all kernel (the unbiased-var factor is folded
                # into gamma host-side; the +1e-6-on-std term is 3e-8 rel).
                rs = fsm.tile([128, 1], f32, tag="rs", name="rs")
                nc.vector.tensor_scalar(out=rs[:], in0=mv[:, 1:2], scalar1=-0.5,
                                        scalar2=None, op0=ALU.pow)
                nc.vector.tensor_scalar(out=work[:], in0=src[:],
                                        scalar1=mv[:, 0:1], scalar2=rs[:],
                                        op0=ALU.subtract, op1=ALU.mult)
                nc.vector.tensor_mul(work[:], work[:], g_bc[:])
                nc.vector.tensor_add(dst[:], work[:], b_bc[:])

            out1s = {}

            def ffn_front(g):
                if sim_single:
                    nc.sync.dma_start(a2a_out[g][:], a2a_in[g][:])
                else:
                    nc.gpsimd.collective_compute(
                        "AllToAll", ALU.bypass,
                        replica_groups=[list(range(NCORES))],
                        ins=[a2a_in[g].opt()], outs=[a2a_out[g].opt()])
                # assemble attention-out tile: rows 0:64 batch-0 block,
                # rows 64:128 batch-1 block; src r covers dims 256*(r%4)
                at = fsb.tile([128, H], f16, tag="ta", name="at", bufs=1)
                for bh in range(2):
                    nc.scalar.dma_start(
                        at[64 * bh:64 * (bh + 1), :].rearrange(
                            "p (r d) -> p r d", r=4),
                        a2a_out[g][4 * bh:4 * (bh + 1), :, :]
                        .rearrange("r p d -> p r d"))
                xr = fsm.tile([128, H], f16, tag="xr", name="xr", bufs=1)
                nc.scalar.dma_start(xr[:], ap["xres"][128 * g:128 * (g + 1), :])
                nc.vector.tensor_add(at[:], at[:], xr[:])
                out1 = fsb.tile([128, H], f16, tag="o1", name="out1")
                wk_ = fsb.tile([128, H], f16, tag="td", name="wk_", bufs=1)
                layer_norm_to(out1, at, lnbc["g1"], lnbc["b1"], wk_)
                out1s[g] = out1

            def ffn_back(g):
                out1 = out1s.pop(g)
                # transpose LN1 out for the H-contraction of FFN1
                out1T = fsb.tile([128, KCH, 128], f16, tag="o1T", name="out1T",
                                 bufs=1)
                for a in range(KCH):
                    tp = psm.tile([128, 128], f16, tag="mt", name="tp")
                    nc.tensor.transpose(
                        tp[:], out1[:, 128 * a:128 * (a + 1)], identh[:])
                    nc.gpsimd.tensor_copy(out1T[:, a, :], tp[:])
                # FFN1: ht[f, t] = relu(W1^T x out1T) for this 128-token tile
                ht = fsb.tile([128, FF // 128, 128], f16, tag="ht", name="ht",
                              bufs=1)
                for ft in range(FF // 128):
                    hp_ = psf.tile([128, 128], f32, tag="hp", name="hp", bufs=1)
                    for a in range(KCH):
                        nc.tensor.matmul(
                            hp_[:], w1t[:, a, 128 * ft:128 * (ft + 1)],
                            out1T[:, a, :], start=(a == 0), stop=(a == KCH - 1))
                    nc.gpsimd.tensor_scalar_max(ht[:, ft, :], hp_[:], 0.0)
                # FFN2 + fused LN2 tail
                h2 = fsb.tile([128, H], f16, tag="tc", name="h2", bufs=1)
                for oc in range(2):
                    acc = pss.tile([128, 512], f32, tag="st", name="o2acc")
                    for ft in range(FF // 128):
                        nc.tensor.matmul(
                            acc[:], ht[:, ft, :],
                            w2t[:, ft, 512 * oc:512 * (oc + 1)],
                            start=(ft == 0), stop=(ft == FF // 128 - 1))
                    nc.gpsimd.tensor_copy(h2[:, 512 * oc:512 * (oc + 1)], acc[:])
                nc.vector.tensor_add(h2[:], h2[:], out1[:])
                fin = fsb.tile([128, H], f16, tag="tb2", name="fin", bufs=1)
                wk2 = fsb.tile([128, H], f16, tag="td", name="wk2", bufs=1)
                layer_norm_to(fin, h2, lnbc["g2"], lnbc["b2"], wk2)
                nc.scalar.dma_start(out_ap[128 * g:128 * (g + 1), :], fin[:])

            # ---------------- main loop: g-outer units; a1(i+2) interleaved
            # into b(i); a2a+LN1 for group g emitted right after its last
            # head, the PE-heavy FFN back-half 3 units later ----------------
            units = [(h, g) for g in range(NG) for h in range(HPC)]
            LOOKAHEAD = 2
            pend = {}
            for k in range(LOOKAHEAD):
                ms, gen = stage_a1_gen(*units[k])
                for _ in range(16):
                    next(gen)
                stage_a2(*units[k], ms)
            for i, (h, g) in enumerate(units):
                j = i + LOOKAHEAD
                if j < len(units):
                    pend[j] = stage_a1_gen(*units[j])
                stage_b(h, g, pend[j][1] if j in pend else None)
                if j < len(units):
                    stage_a2(*units[j], pend.pop(j)[0])
                if i % HPC == 3:
                    ffn_front(i // HPC)
                if i % HPC == 2 and i // HPC >= 1:
                    ffn_back(i // HPC - 1)
            ffn_back(NG - 1)

    nc.compile()
    if not sim_single:
        nc.m = get_hw_module(nc.m)
    return nc


_NC_CACHE = {}


def _get_program():
    if "nc" not in _NC_CACHE:
        _NC_CACHE["nc"] = _build_program()
    return _NC_CACHE["nc"]


def _prep_inputs(x, Wqkv, bqkv, W1, b1, W2, b2, gamma1, beta1, gamma2, beta2):
    """Host-side slicing/folding into per-core in_maps."""
    x = np.asarray(x, np.float32)
    Wqkv = np.asarray(Wqkv, np.float32)
    bqkv = np.asarray(bqkv, np.float32)
    d = np.arange(HD)
    hh = np.arange(NH)
    # qkv reshape in reference: [B,T,HD,3,NH] -> col = d*48 + k*16 + h
    cols = d[:, None, None] * (3 * NH) + np.arange(3)[None, :, None] * NH \
        + hh[None, None, :]
    Wq = Wqkv[:, cols[:, 0, :]] * (bqkv[cols[:, 0, :]] / np.sqrt(H))[None]
    Wk = Wqkv[:, cols[:, 1, :]] * bqkv[cols[:, 1, :]][None]
    Wv = Wqkv[:, cols[:, 2, :]] * bqkv[cols[:, 2, :]][None]
    # -> [H, HD, NH]; per-core head-major layout [H, 4*HD] (head-local major)
    Wq = np.transpose(Wq, (0, 2, 1)).astype(np.float16)  # [H, NH, HD]
    Wk = np.transpose(Wk, (0, 2, 1)).astype(np.float16)
    Wv = np.transpose(Wv, (0, 2, 1)).astype(np.float16)
    W1e = (np.asarray(W1, np.float32) * np.asarray(b1, np.float32)[None]) \
        .astype(np.float16)
    W2e = (np.asarray(W2, np.float32) * np.asarray(b2, np.float32)[None]) \
        .astype(np.float16)
    # rs is computed as 1/std_biased (exp(-0.5*ln(var))); fold the unbiased
    # correction sqrt((H-1)/H) into gamma
    gc = np.sqrt((H - 1.0) / H)
    lnw = np.stack([gamma1 * gc, beta1, gamma2 * gc, beta2]).astype(np.float16)
    xT = [np.ascontiguousarray(x[b].T).astype(np.float16) for b in range(B)]
    in_maps = []
    for c in range(NCORES):
        b, grp = c // 4, c % 4
        heads = slice(4 * grp, 4 * grp + 4)
        # FFN tokens of core c: per group g, batch b': x[b', 512g+64c : +64]
        xres = np.empty((TOK, H), np.float16)
        for g in range(NG):
            t0 = 512 * g + 64 * c
            xres[128 * g:128 * g + 64] = x[0, t0:t0 + 64]
            xres[128 * g + 64:128 * g + 128] = x[1, t0:t0 + 64]
        in_maps.append({
            "xT": xT[b],
            "xres": xres,
            "wq": np.ascontiguousarray(Wq[:, heads, :].reshape(H, 4 * HD)),
            "wk": np.ascontiguousarray(Wk[:, heads, :].reshape(H, 4 * HD)),
            "wv": np.ascontiguousarray(Wv[:, heads, :].reshape(H, 4 * HD)),
            "w1": W1e, "w2": W2e, "lnw": lnw,
        })
    return in_maps


def kernel(x, Wqkv, bqkv, W1, b1, W2, b2, gamma1, beta1, gamma2, beta2,
           _trace=False):
    nc = _get_program()
    in_maps = _prep_inputs(x, Wqkv, bqkv, W1, b1, W2, b2,
                           gamma1, beta1, gamma2, beta2)
    res = run_bass_kernel_spmd(nc, in_maps, core_ids=list(range(NCORES)),
                               trace=_trace)
    out = np.empty((B, T, H), np.float32)
    for c in range(NCORES):
        oc = res.results[c]["out"]  # [512, H]: (g, batch-half, 64)
        for g in range(NG):
            t0 = 512 * g + 64 * c
            out[0, t0:t0 + 64] = oc[128 * g:128 * g + 64]
            out[1, t0:t0 + 64] = oc[128 * g + 64:128 * g + 128]
    if _trace:
        kernel.last_results = res
    return out


# revision 37
# speedup vs baseline: 1.3460x; 1.0247x over previous
"""TRN2 Bass kernel for nn_DecoderLayer_47175920779446.

Full decoder layer: qkv (mul-bias) -> 16-head attention -> +res -> LN ->
FFN(relu, mul-bias) -> +res -> LN, on x[2, 2048, 1024] fp32.

Sharding (8 cores): attention is sharded by (batch, 4 heads): core c handles
batch c//4, heads 4*(c%4)..4*(c%4)+3 over all 2048 tokens of its batch.
FFN/LN are sharded by strided 64-token blocks: core c owns blocks
{t//64 == 8*m + c} of BOTH batches (256+256 tokens). Attention runs
query-group-outer; after each group g finishes (4 heads), a small fp16
AllToAll chunk (8 x [8, 64, 256]) reshards that group's attention output,
and the full FFN chain for that 128-token tile (LN1 -> FFN1 -> FFN2 -> LN2)
runs interleaved under the next attention group's matmuls.

Precision: scores run in fp16 (11-bit mantissa, matching the fp16 projection
noise floor ~2e-4*sigma): S = q16*k16 + m_hat (K=65 with a fused bias row).
V/P/FFN run fp16; LN stats and residual sums run fp32 where it matters.
"""
import contextlib
import numpy as np

import concourse.bass as bass
import concourse.tile as tile
from concourse import bacc, mybir
from concourse.bass_utils import run_bass_kernel_spmd
from concourse.bass_interp import get_hw_module
from concourse.masks import make_identity

H, NH, HD, FF = 1024, 16, 64, 4096
B, T = 2, 2048
EPS = 1e-6
NCORES = 8
HPC = NH // 4          # 4 heads per core
TOK = (B * T) // NCORES  # 512 tokens per core through the FFN
NKC = T // 128         # 16 key chunks
NG = T // 512          # 4 query groups
KCH = H // 128         # 8 contraction chunks for qkv
f32, f32r, bf16 = mybir.dt.float32, mybir.dt.float32r, mybir.dt.bfloat16
f16 = mybir.dt.float16
AF = mybir.ActivationFunctionType
ALU = mybir.AluOpType


def _build_program(sim_single=False):
    nc = bacc.Bacc("TRN2", target_bir_lowering=False, debug=False,
                   num_devices=1 if sim_single else NCORES)
    ap = {}
    ap["xT"] = nc.dram_tensor("xT", [H, T], f16, kind="ExternalInput").ap()
    ap["xres"] = nc.dram_tensor("xres", [TOK, H], f16, kind="ExternalInput").ap()
    for w in ("wq", "wk", "wv"):
        ap[w] = nc.dram_tensor(w, [H, 4 * HD], f16, kind="ExternalInput").ap()
    ap["w1"] = nc.dram_tensor("w1", [H, FF], f16, kind="ExternalInput").ap()
    ap["w2"] = nc.dram_tensor("w2", [FF, H], f16, kind="ExternalInput").ap()
    ap["lnw"] = nc.dram_tensor("lnw", [4, H], f16, kind="ExternalInput").ap()
    out_ap = nc.dram_tensor("out", [TOK, H], f16, kind="ExternalOutput").ap()

    with tile.TileContext(nc) as tc:
        ctx = contextlib.ExitStack()
        with ctx:
            const = ctx.enter_context(tc.tile_pool(name="const", bufs=1))
            dram = ctx.enter_context(tc.tile_pool(name="dram", bufs=1, space="DRAM"))

            identh = const.tile([128, 128], f16)
            make_identity(nc, identh[:])

            # per-group a2a chunks: slot r holds 64-token block (t//64 == 8g+r)
            # of this core's batch, its 4 heads (256 dims), fp16.
            a2a_in = [dram.tile([NCORES, 64, 4 * HD], f16, name=f"a2a_in{g}")
                      for g in range(NG)]
            a2a_out = [dram.tile([NCORES, 64, 4 * HD], f16, name=f"a2a_out{g}")
                       for g in range(NG)]

            # score/softmax operand pools (live through all attention units)
            qk = ctx.enter_context(tc.tile_pool(name="qk", bufs=1))
            sb = ctx.enter_context(tc.tile_pool(name="sb", bufs=3))
            small = ctx.enter_context(tc.tile_pool(name="small", bufs=4))
            psn = ctx.enter_context(tc.tile_pool(name="psn", bufs=2, space="PSUM"))
            pss = ctx.enter_context(tc.tile_pool(name="pss", bufs=2, space="PSUM"))
            pso = ctx.enter_context(tc.tile_pool(name="pso", bufs=1, space="PSUM"))
            psm = ctx.enter_context(tc.tile_pool(name="psm", bufs=2, space="PSUM"))

            til_q, til_k = {}, {}
            for h in range(HPC):
                til_q[h] = qk.tile([65, T], f16, name=f"til_q{h}", tag="tq", bufs=HPC)
                til_k[h] = qk.tile([65, T], f16, name=f"til_k{h}", tag="tk", bufs=HPC)
                nc.gpsimd.memset(til_k[h][64:65, :], 1.0)
            vn = []
            for kc in range(NKC):
                v = qk.tile([128, HPC, 65], f16, name=f"vn{kc}", tag="vn", bufs=NKC)
                nc.gpsimd.memset(v[:, :, 64:65], 1.0)
                vn.append(v)

            # ---------------- QKV projections (own scope: frees w/x tiles) --
            qctx = contextlib.ExitStack()
            with qctx:
                wpool = qctx.enter_context(tc.tile_pool(name="wpool", bufs=1))
                xgp = qctx.enter_context(tc.tile_pool(name="xgp", bufs=4))

                w_sb, xgs = {}, []

                def load_w(w):
                    w_sb[w] = wpool.tile([128, KCH, 4 * HD], f16, name=f"sb_{w}")
                    nc.sync.dma_start(
                        w_sb[w][:], ap[w].rearrange("(a p) c -> p a c", p=128))

                def load_xg(g):
                    gsl = slice(512 * g, 512 * (g + 1))
                    xg = xgp.tile([128, KCH, 512], f16, name=f"xg{g}", tag="xg", bufs=4)
                    nc.scalar.dma_start(
                        xg[:], ap["xT"].rearrange("(a p) t -> p a t", p=128)[:, :, gsl])
                    xgs.append(xg)

                load_w("wk")
                load_xg(0)
                load_w("wq")
                load_w("wv")
                for g in range(1, NG):
                    load_xg(g)

                def proj_pass(name, til, g):
                    gsl = slice(512 * g, 512 * (g + 1))
                    for hp in range(2):  # head pairs
                        p = pss.tile([128, 512], f32, tag="st", name="pqk")
                        for a in range(KCH):
                            nc.tensor.matmul(
                                p[:], w_sb[name][:, a, 128 * hp:128 * (hp + 1)],
                                xgs[g][:, a, :], start=(a == 0), stop=(a == KCH - 1))
                        for hl in range(2):
                            h = 2 * hp + hl
                            rows = slice(64 * hl, 64 * (hl + 1))
                            nc.scalar.activation(til[h][0:64, gsl], p[rows, :], AF.Copy)

                for g in range(NG):
                    proj_pass("wk", til_k, g)
                for g in range(NG):
                    proj_pass("wq", til_q, g)
                    for tt in range(4):  # V natural per token tile
                        kc = 4 * g + tt
                        p = pss.tile([128, 4 * HD], f32, tag="st", name="pv")
                        for a in range(KCH):
                            nc.tensor.matmul(
                                p[:], xgs[g][:, a, 128 * tt:128 * (tt + 1)],
                                w_sb["wv"][:, a, :], start=(a == 0), stop=(a == KCH - 1))
                        nc.scalar.activation(
                            vn[kc][:, :, 0:64],
                            p[:].rearrange("p (h d) -> p h d", h=HPC), AF.Copy)

            # ---------------- FFN pools (reuse qkv space; weights stream
            # during attention) ----------------
            w1p = ctx.enter_context(tc.tile_pool(name="w1p", bufs=1))
            w2p = ctx.enter_context(tc.tile_pool(name="w2p", bufs=1))
            fsb = ctx.enter_context(tc.tile_pool(name="fsb", bufs=2))
            o1p = ctx.enter_context(tc.tile_pool(name="o1p", bufs=1))
            fsm = ctx.enter_context(tc.tile_pool(name="fsm", bufs=4))
            psf = ctx.enter_context(tc.tile_pool(name="psf", bufs=1, space="PSUM"))

            w1t = w1p.tile([128, KCH, FF], f16, name="w1t")
            for c in range(8):
                nc.sync.dma_start(
                    w1t[:, :, 512 * c:512 * (c + 1)],
                    ap["w1"].rearrange("(a p) f -> p a f", p=128)
                    [:, :, 512 * c:512 * (c + 1)])
            w2t = w2p.tile([128, FF // 128, H], f16, name="w2t")
            for c in range(8):
                nc.sync.dma_start(
                    w2t[:, 4 * c:4 * (c + 1), :],
                    ap["w2"].rearrange("(a p) o -> p a o", p=128)
                    [:, 4 * c:4 * (c + 1), :])

            lnbc = {}
            for i, nm in enumerate(("g1", "b1", "g2", "b2")):
                lnbc[nm] = o1p.tile([128, H], f16, name=f"ln_{nm}", tag="lnbc", bufs=4)
                nc.sync.dma_start(
                    lnbc[nm][:], ap["lnw"][i, :].partition_broadcast(128))


            # ---------------- attention stages ----------------
            def stage_a1_gen(h, g):
                # natural-S matmuls + DVE max reduces, yielded one half-step
                # at a time so stage_b emission can interleave them (avoids
                # rate-limiting PE on the 2-slot sn ring)
                mstage = small.tile([128, 4], f16, tag="mstage", name="mstage",
                                    bufs=3)

                def gen():
                    for qt in range(4):
                        qsl = slice(512 * g + 128 * qt, 512 * g + 128 * (qt + 1))
                        negmax = []
                        for half in range(4):
                            sn = psn.tile([128, 512], f32, name="sn")
                            ks = slice(512 * half, 512 * (half + 1))
                            nc.tensor.matmul(
                                sn[:], til_q[h][0:64, qsl], til_k[h][0:64, ks],
                                start=True, stop=True)
                            nm = small.tile([128, 1], f32, tag="nm", name="nm",
                                            bufs=8)
                            nc.vector.tensor_reduce(
                                nm[:], sn[:], axis=mybir.AxisListType.X,
                                op=ALU.max, negate=True)
                            negmax.append(nm)
                            yield
                        nc.vector.tensor_tensor(
                            negmax[0][:], negmax[0][:], negmax[1][:], ALU.min)
                        nc.vector.tensor_tensor(
                            negmax[2][:], negmax[2][:], negmax[3][:], ALU.min)
                        nc.vector.tensor_tensor(
                            mstage[:, qt:qt + 1], negmax[0][:], negmax[2][:],
                            ALU.min)
                    while True:
                        yield

                return mstage, gen()

            def stage_a2(h, g, mstage):
                # emitted a period later so the PE transpose never waits on DVE
                for qt in range(4):
                    qsl = slice(512 * g + 128 * qt, 512 * g + 128 * (qt + 1))
                    mt = psm.tile([1, 128], f16, tag="mt", name="mt")
                    nc.tensor.transpose(mt[:], mstage[:, qt:qt + 1], identh[:])
                    nc.gpsimd.tensor_copy(til_q[h][64:65, qsl], mt[:])

            def stage_b(h, g, inter=None):
                gsl = slice(512 * g, 512 * (g + 1))
                o_acc = pso.tile([65, 512], f32, name="o_acc")
                pts = {}
                PVLAG = 2

                def pv(kc):
                    nc.tensor.matmul(o_acc[:], vn[kc][:, h, :], pts.pop(kc)[:],
                                     start=(kc == 0), stop=(kc == NKC - 1))

                for kc in range(NKC):
                    if inter is not None:
                        next(inter)
                    ksl = slice(128 * kc, 128 * (kc + 1))
                    st = pss.tile([128, 512], f32, tag="st", name="st")
                    nc.tensor.matmul(st[:], til_k[h][0:65, ksl],
                                     til_q[h][0:65, gsl], start=True, stop=True)
                    pt = sb.tile([128, 512], f16, tag="pt", name="pt", bufs=3)
                    nc.scalar.activation(pt[:], st[:], AF.Exp)
                    pts[kc] = pt
                    if kc >= PVLAG:
                        pv(kc - PVLAG)
                for kc in range(NKC - PVLAG, NKC):
                    pv(kc)
                ot = sb.tile([65, 512], f16, tag="ot", name="ot", bufs=2)
                nc.gpsimd.tensor_copy(ot[:], o_acc[:])
                # transpose to natural, scale by 1/denom, ship to a2a chunk g:
                # token rows 0:64 -> slot 2*tt, rows 64:128 -> slot 2*tt+1;
                # single batched DMA per unit (SP sequencer time is 565ns per
                # dma_start -- instruction count matters)
                obu = sb.tile([128, 4, HD], f16, tag="ob", name="obu", bufs=2)
                for tt in range(4):
                    op_ = psm.tile([128, 65], f16, tag="mt", name="opt")
                    nc.tensor.transpose(
                        op_[:], ot[0:65, 128 * tt:128 * (tt + 1)],
                        identh[0:65, 0:65])
                    rc = small.tile([128, 1], f32, tag="rc", name="rc")
                    nc.vector.reciprocal(rc[:], op_[:, 64:65])
                    nc.gpsimd.tensor_scalar_mul(obu[:, tt, :], op_[:, 0:64], rc[:])
                nc.sync.dma_start(
                    a2a_in[g][:, :, 64 * h:64 * (h + 1)]
                    .rearrange("(q s) p d -> (s p) q d", s=2),
                    obu[:])

            # ---------------- per-chunk FFN chain ----------------
            def layer_norm_to(dst, src, g_bc, b_bc, work):
                """dst = gamma*(src-mean)/(std_unbiased+EPS)+beta, [128,H]."""
                stats = fsm.tile([128, 2, 6], f32, tag="stats", name="stats")
                for hf in range(2):
                    nc.vector.bn_stats(stats[:, hf, :],
                                       src[:, 512 * hf:512 * (hf + 1)])
                mv = fsm.tile([128, 2], f32, tag="mv", name="mv")
                nc.vector.bn_aggr(mv[:], stats[:])
                # 1/std via DVE pow(var, -0.5): keeps the ACT engine on the
                # Exp table set all kernel (the unbiased-var factor is folded
                # into gamma host-side; the +1e-6-on-std term is 3e-8 rel).
                rs = fsm.tile([128, 1], f32, tag="rs", name="rs")
                nc.vector.tensor_scalar(out=rs[:], in0=mv[:, 1:2], scalar1=-0.5,
                                        scalar2=None, op0=ALU.pow)
                nc.vector.tensor_scalar(out=work[:], in0=src[:],
                                        scalar1=mv[:, 0:1], scalar2=rs[:],
                                        op0=ALU.subtract, op1=ALU.mult)
                nc.vector.tensor_mul(work[:], work[:], g_bc[:])
                nc.vector.tensor_add(dst[:], work[:], b_bc[:])

            out1s = {}

            def ffn_front(g):
                if sim_single:
                    nc.sync.dma_start(a2a_out[g][:], a2a_in[g][:])
                else:
                    nc.gpsimd.collective_compute(
                        "AllToAll", ALU.bypass,
                        replica_groups=[list(range(NCORES))],
                        ins=[a2a_in[g].opt()], outs=[a2a_out[g].opt()])
                # assemble attention-out tile: rows 0:64 batch-0 block,
                # rows 64:128 batch-1 block; src r covers dims 256*(r%4)
                xr = fsm.tile([128, H], f16, tag="xr", name="xr", bufs=1)
                nc.scalar.dma_start(xr[:], ap["xres"][128 * g:128 * (g + 1), :])
                at = fsb.tile([128, H], f16, tag="ta", name="at", bufs=1)
                for bh in range(2):
                    nc.scalar.dma_start(
                        at[64 * bh:64 * (bh + 1), :].rearrange(
                            "p (r d) -> p r d", r=4),
                        a2a_out[g][4 * bh:4 * (bh + 1), :, :]
                        .rearrange("r p d -> p r d"))
                nc.vector.tensor_add(at[:], at[:], xr[:])
                out1 = fsb.tile([128, H], f16, tag="o1", name="out1")
                wk_ = fsb.tile([128, H], f16, tag="td", name="wk_", bufs=1)
                layer_norm_to(out1, at, lnbc["g1"], lnbc["b1"], wk_)
                out1s[g] = out1

            def ffn_back(g):
                out1 = out1s.pop(g)
                # transpose LN1 out for the H-contraction of FFN1
                out1T = fsb.tile([128, KCH, 128], f16, tag="o1T", name="out1T",
                                 bufs=1)
                for a in range(KCH):
                    tp = psm.tile([128, 128], f16, tag="mt", name="tp")
                    nc.tensor.transpose(
                        tp[:], out1[:, 128 * a:128 * (a + 1)], identh[:])
                    nc.scalar.activation(out1T[:, a, :], tp[:], AF.Copy)
                # FFN1: ht[f, t] = relu(W1^T x out1T) for this 128-token tile
                ht = fsb.tile([128, FF // 128, 128], f16, tag="ht", name="ht",
                              bufs=1)
                for ft in range(FF // 128):
                    hp_ = psf.tile([128, 128], f32, tag="hp", name="hp", bufs=1)
                    for a in range(KCH):
                        nc.tensor.matmul(
                            hp_[:], w1t[:, a, 128 * ft:128 * (ft + 1)],
                            out1T[:, a, :], start=(a == 0), stop=(a == KCH - 1))
                    nc.gpsimd.tensor_scalar_max(ht[:, ft, :], hp_[:], 0.0)
                # FFN2 + fused LN2 tail
                h2 = fsb.tile([128, H], f16, tag="tc", name="h2", bufs=1)
                for oc in range(2):
                    acc = pss.tile([128, 512], f32, tag="st", name="o2acc")
                    for ft in range(FF // 128):
                        nc.tensor.matmul(
                            acc[:], ht[:, ft, :],
                            w2t[:, ft, 512 * oc:512 * (oc + 1)],
                            start=(ft == 0), stop=(ft == FF // 128 - 1))
                    nc.scalar.activation(h2[:, 512 * oc:512 * (oc + 1)], acc[:], AF.Copy)
                nc.vector.tensor_add(h2[:], h2[:], out1[:])
                fin = fsb.tile([128, H], f16, tag="tb2", name="fin", bufs=1)
                wk2 = fsb.tile([128, H], f16, tag="td", name="wk2", bufs=1)
                layer_norm_to(fin, h2, lnbc["g2"], lnbc["b2"], wk2)
                nc.scalar.dma_start(out_ap[128 * g:128 * (g + 1), :], fin[:])

            # ---------------- main loop: g-outer units; a1(i+2) interleaved
            # into b(i); a2a+LN1 for group g emitted right after its last
            # head, the PE-heavy FFN back-half 3 units later ----------------
            units = [(h, g) for g in range(NG) for h in range(HPC)]
            LOOKAHEAD = 2
            pend = {}
            for k in range(LOOKAHEAD):
                ms, gen = stage_a1_gen(*units[k])
                for _ in range(16):
                    next(gen)
                stage_a2(*units[k], ms)
            for i, (h, g) in enumerate(units):
                j = i + LOOKAHEAD
                if j < len(units):
                    pend[j] = stage_a1_gen(*units[j])
                stage_b(h, g, pend[j][1] if j in pend else None)
                if j < len(units):
                    stage_a2(*units[j], pend.pop(j)[0])
                if i % HPC == 3:
                    ffn_front(i // HPC)
                if i % HPC == 2 and i // HPC >= 1:
                    ffn_back(i // HPC - 1)
            ffn_back(NG - 1)

    nc.compile()
    if not sim_single:
        nc.m = get_hw_module(nc.m)
    return nc


_NC_CACHE = {}


def _get_program():
    if "nc" not in _NC_CACHE:
        _NC_CACHE["nc"] = _build_program()
    return _NC_CACHE["nc"]


def _prep_inputs(x, Wqkv, bqkv, W1, b1, W2, b2, gamma1, beta1, gamma2, beta2):
    """Host-side slicing/folding into per-core in_maps."""
    x = np.asarray(x, np.float32)
    Wqkv = np.asarray(Wqkv, np.float32)
    bqkv = np.asarray(bqkv, np.float32)
    d = np.arange(HD)
    hh = np.arange(NH)
    # qkv reshape in reference: [B,T,HD,3,NH] -> col = d*48 + k*16 + h
    cols = d[:, None, None] * (3 * NH) + np.arange(3)[None, :, None] * NH \
        + hh[None, None, :]
    Wq = Wqkv[:, cols[:, 0, :]] * (bqkv[cols[:, 0, :]] / np.sqrt(H))[None]
    Wk = Wqkv[:, cols[:, 1, :]] * bqkv[cols[:, 1, :]][None]
    Wv = Wqkv[:, cols[:, 2, :]] * bqkv[cols[:, 2, :]][None]
    # -> [H, HD, NH]; per-core head-major layout [H, 4*HD] (head-local major)
    Wq = np.transpose(Wq, (0, 2, 1)).astype(np.float16)  # [H, NH, HD]
    Wk = np.transpose(Wk, (0, 2, 1)).astype(np.float16)
    Wv = np.transpose(Wv, (0, 2, 1)).astype(np.float16)
    W1e = (np.asarray(W1, np.float32) * np.asarray(b1, np.float32)[None]) \
        .astype(np.float16)
    W2e = (np.asarray(W2, np.float32) * np.asarray(b2, np.float32)[None]) \
        .astype(np.float16)
    # rs is computed as 1/std_biased (exp(-0.5*ln(var))); fold the unbiased
    # correction sqrt((H-1)/H) into gamma
    gc = np.sqrt((H - 1.0) / H)
    lnw = np.stack([gamma1 * gc, beta1, gamma2 * gc, beta2]).astype(np.float16)
    xT = [np.ascontiguousarray(x[b].T).astype(np.float16) for b in range(B)]
    in_maps = []
    for c in range(NCORES):
        b, grp = c // 4, c % 4
        heads = slice(4 * grp, 4 * grp + 4)
        # FFN tokens of core c: per group g, batch b': x[b', 512g+64c : +64]
        xres = np.empty((TOK, H), np.float16)
        for g in range(NG):
            t0 = 512 * g + 64 * c
            xres[128 * g:128 * g + 64] = x[0, t0:t0 + 64]
            xres[128 * g + 64:128 * g + 128] = x[1, t0:t0 + 64]
        in_maps.append({
            "xT": xT[b],
            "xres": xres,
            "wq": np.ascontiguousarray(Wq[:, heads, :].reshape(H, 4 * HD)),
            "wk": np.ascontiguousarray(Wk[:, heads, :].reshape(H, 4 * HD)),
            "wv": np.ascontiguousarray(Wv[:, heads, :].reshape(H, 4 * HD)),
            "w1": W1e, "w2": W2e, "lnw": lnw,
        })
    return in_maps


def kernel(x, Wqkv, bqkv, W1, b1, W2, b2, gamma1, beta1, gamma2, beta2,
           _trace=False):
    nc = _get_program()
    in_maps = _prep_inputs(x, Wqkv, bqkv, W1, b1, W2, b2,
                           gamma1, beta1, gamma2, beta2)
    res = run_bass_kernel_spmd(nc, in_maps, core_ids=list(range(NCORES)),
                               trace=_trace)
    out = np.empty((B, T, H), np.float32)
    for c in range(NCORES):
        oc = res.results[c]["out"]  # [512, H]: (g, batch-half, 64)
        for g in range(NG):
            t0 = 512 * g + 64 * c
            out[0, t0:t0 + 64] = oc[128 * g:128 * g + 64]
            out[1, t0:t0 + 64] = oc[128 * g + 64:128 * g + 128]
    if _trace:
        kernel.last_results = res
    return out
